# revision 1
# baseline (speedup 1.0000x reference)
"""DiffSeg segmentation head on 8 Trainium2 NeuronCores (Bass/Tile).

Pipeline (per image, B=1):
  L1 (device): multi-scale aggregation of attention maps -> agg [4096,4096] f32
      (row-sharded 512 rows/core), anchor gather + symmetric-KL merge0
      (fp16 log/matmuls like the reference), ReduceScatter -> X2 p-slices,
      partial C1 = X2 @ lX2.T for the first greedy KL matrix.
  host: assemble klmat2, run the sequential greedy selection (bitsets).
  L2 (device): new2 = sel2 @ X2 / cnt, partial C3 for the second greedy.
  host: greedy again.
  L3 (device): new3 = sel3 @ new2 / cnt, 4x bilinear upsample (align_corners),
      invalid-row bias, transpose, per-pixel argmax over anchors.

Self-contained: hardcodes shapes/sharding for inputs
  weight_64 [B,8,4096,4096], weight_32 [B,8,1024,1024],
  weight_16 [B,8,256,256],   weight_8  [B,8,64,64]  (B=1)
"""

import sys
import numpy as np

for _p in ("/opt/trn_rl_repo", "/opt/trn_rl_repo/concourse"):
    if _p not in sys.path:
        sys.path.append(_p)

NCORES = 8
THR2 = np.float32(1.8)          # == 2 * f32(0.9), exact
RAT = [np.float32(64.0 / 120.0), np.float32(32.0 / 120.0),
       np.float32(16.0 / 120.0), np.float32(8.0 / 120.0)]
NEG_BIG = np.float32(-1e38)


def _up_coords(n, r):
    s = np.linspace(0.0, n - 1.0, n * r)
    i0 = np.clip(np.floor(s).astype(np.int64), 0, n - 2)
    w = (s - i0).astype(np.float32)
    return i0.astype(np.int64), w


# ---------------------------------------------------------------- host consts
X032, W32 = _up_coords(32, 2)      # 32 -> 64
X016, W16 = _up_coords(16, 4)
X08, W8 = _up_coords(8, 8)
Y0U, WYU = _up_coords(64, 4)       # 64 -> 256 (final upsample)
X0U, WXU = _up_coords(64, 4)

# L3 per-core y windows
L3_LO = [int(Y0U[32 * k]) for k in range(NCORES)]          # window start row
L3_W = 12  # padded window size (12 = smallest mult-of-4 >= 10)

ANCHOR_PTS = [1 + 4 * i for i in range(16)]


def _wrap_idx(vals):
    """ap_gather index layout: j -> partition j%16, elem j//16; replicated to
    all 8 gpsimd cores (16-partition groups)."""
    n = len(vals)
    cols = (n + 15) // 16
    arr = np.zeros((16, cols), np.int16)
    for j, v in enumerate(vals):
        arr[j % 16, j // 16] = v
    return np.tile(arr, (8, 1))


def _l3_host_consts():
    consts = []
    # x-gather indices (uniform across cores): view [32y*64x]; idx = y*64+x0(j)
    idxx_c = np.array([y * 64 + X0U[j] for y in range(32) for j in range(256)],
                      np.int16)
    idxx_d = np.array([y * 64 + X0U[j] + 1 for y in range(32) for j in range(256)],
                      np.int16)
    wxr = np.broadcast_to(WXU[None, :], (32, 256)).reshape(1, -1).astype(np.float32)
    for k in range(NCORES):
        lo = L3_LO[k]
        y0l = [int(Y0U[32 * k + t]) - lo for t in range(32)]
        consts.append(dict(
            idxyc=_wrap_idx(np.array(y0l, np.int16)),
            idxyd=_wrap_idx(np.array(y0l, np.int16) + 1),
            wy=WYU[32 * k:32 * k + 32][None, :].astype(np.float32),
            idxxc=_wrap_idx(idxx_c),
            idxxd=_wrap_idx(idxx_d),
            wx=WXU[None, :].astype(np.float32),
        ))
    return consts


L3C = _l3_host_consts()
_IWIN_CAT = np.concatenate(
    [_wrap_idx(np.clip(np.arange(L3_LO[k], L3_LO[k] + L3_W), 0, 63)
               .astype(np.int16)) for k in range(NCORES)], axis=0)
_L3_DEV = {}


def _l3_const_dev():
    """Constant L3 inputs, concatenated and staged on device once."""
    if not _L3_DEV:
        import jax
        from jax.sharding import Mesh, PartitionSpec, NamedSharding
        mesh = Mesh(np.asarray(jax.devices()[:NCORES]), ("core",))
        sh = NamedSharding(mesh, PartitionSpec("core"))
        for nm in ("wy", "wx", "idxyc", "idxyd", "idxxc", "idxxd"):
            cat = np.concatenate([L3C[k][nm] for k in range(NCORES)], axis=0)
            _L3_DEV[nm] = jax.device_put(cat, sh)
    return _L3_DEV

# ------------------------------------------------------------------- programs
_PROGS = {}


def _mybir():
    from concourse import mybir
    return mybir


def _build_common():
    import concourse.tile as tile
    from concourse import mybir, bacc
    from concourse.bass_utils import axon_active
    nc = bacc.Bacc("TRN2", target_bir_lowering=False, debug=False,
                   enable_asserts=False, num_devices=NCORES)
    return nc, tile, mybir



def _build_l1():
    nc, tile, mybir = _build_common()
    from concourse.masks import make_identity
    F32, F16 = mybir.dt.float32, mybir.dt.float16
    ALU, AX, AF = mybir.AluOpType, mybir.AxisListType, mybir.ActivationFunctionType
    RG = [list(range(NCORES))]

    w64 = nc.dram_tensor("w64s", [8, 512, 4096], F32, kind="ExternalInput")
    w32 = nc.dram_tensor("w32s", [8, 256, 1024], F32, kind="ExternalInput")
    w16 = nc.dram_tensor("w16s", [8, 128, 256], F32, kind="ExternalInput")
    w8 = nc.dram_tensor("w8s", [8, 64, 64], F32, kind="ExternalInput")
    o_x2 = nc.dram_tensor("x2slice", [256, 512], F32, kind="ExternalOutput")
    o_kl = nc.dram_tensor("klpack", [257, 256], F32, kind="ExternalOutput")

    with tile.TileContext(nc) as tc:
        with tc.tile_pool(name="consts", bufs=1) as cpool, \
             tc.tile_pool(name="ystore", bufs=1) as ystore, \
             tc.tile_pool(name="dram", bufs=1, space="DRAM") as dram:

            ident16 = cpool.tile([128, 128], F16, tag="id16", name="id16")
            make_identity(nc, ident16[:])
            ident32 = cpool.tile([128, 128], F32, tag="id32", name="id32")
            make_identity(nc, ident32[:])
            ones16 = cpool.tile([128, 1], F16, tag="o16", name="o16")
            nc.gpsimd.memset(ones16[:], 1.0)
            ones32 = cpool.tile([128, 1], F32, tag="o32", name="o32")
            nc.gpsimd.memset(ones32[:], 1.0)

            Y = [ystore.tile([128, 4096], F32, tag=f"Y{pt}", name=f"Y{pt}")
                 for pt in range(4)]

            # ============ phases A+B: aggregation ============
            with tc.tile_pool(name="nmap", bufs=1) as nmap, \
                 tc.tile_pool(name="stage", bufs=2) as stage, \
                 tc.tile_pool(name="workab", bufs=1) as workab:

                def upsample_norm(src_dram, s, P, tidx, out_tile):
                    """head-sum -> bilinear s->64 (x then y) -> normalize."""
                    ssq = s * s
                    m = workab.tile([P, ssq], F32, tag="mA", name="mA")
                    ncb = max(1, ssq // 256)
                    cw = ssq // ncb
                    for cb in range(ncb):
                        stg = stage.tile([P, 8, cw], F32, tag="stg", name="stg")
                        for h in range(8):
                            nc.sync.dma_start(
                                stg[:, h, :],
                                src_dram[h, tidx * P:(tidx + 1) * P,
                                         cb * cw:(cb + 1) * cw])
                        nc.vector.tensor_reduce(
                            m[:, cb * cw:(cb + 1) * cw],
                            stg[:].rearrange("p h x -> p x h"), AX.X, ALU.add)
                    m3 = m[:].rearrange("p (y x) -> p y x", y=s)
                    x0s, wxs = {32: (X032, W32), 16: (X016, W16),
                                8: (X08, W8)}[s]
                    ux = workab.tile([P, s, 64], F32, tag="uxA", name="uxA")
                    tmp = workab.tile([P, s], F32, tag="tmpxA", name="tmpxA")
                    for j in range(64):
                        c = m3[:, :, int(x0s[j])]
                        d = m3[:, :, int(x0s[j]) + 1]
                        eng = nc.vector if j % 2 == 0 else nc.gpsimd
                        eng.tensor_tensor(tmp[:], d, c, ALU.subtract)
                        nc.vector.scalar_tensor_tensor(
                            ux[:, :, j], tmp[:], float(wxs[j]), c,
                            ALU.mult, ALU.add)
                    tmp2 = workab.tile([P, 64], F32, tag="tmpyA", name="tmpyA")
                    for j in range(64):
                        c = ux[:, int(x0s[j]), :]
                        d = ux[:, int(x0s[j]) + 1, :]
                        eng = nc.vector if j % 2 == 0 else nc.gpsimd
                        eng.tensor_tensor(tmp2[:], d, c, ALU.subtract)
                        nc.vector.scalar_tensor_tensor(
                            out_tile[:, j, :], tmp2[:], float(wxs[j]), c,
                            ALU.mult, ALU.add)
                    rs = workab.tile([P, 1], F32, tag="rsA", name="rsA")
                    nc.vector.tensor_reduce(rs[:], out_tile[:], AX.XY, ALU.add)
                    nc.vector.reciprocal(rs[:], rs[:])
                    flat = out_tile[:].rearrange("p a b -> p (a b)")
                    nc.vector.tensor_scalar(flat, flat, rs[:], None, ALU.mult)

                n32 = [nmap.tile([128, 64, 64], F32, tag=f"n32_{t}",
                                 name=f"n32_{t}") for t in range(2)]
                for t in range(2):
                    upsample_norm(w32, 32, 128, t, n32[t])
                n16 = nmap.tile([128, 64, 64], F32, tag="n16", name="n16")
                upsample_norm(w16, 16, 128, 0, n16)
                n8 = nmap.tile([64, 64, 64], F32, tag="n8", name="n8")
                upsample_norm(w8, 8, 64, 0, n8)

                for pt in range(4):
                    for cb in range(8):
                        stg = stage.tile([128, 8, 512], F32, tag="stg",
                                         name="stg64")
                        for h in range(8):
                            nc.sync.dma_start(
                                stg[:, h, :],
                                w64[h, pt * 128:(pt + 1) * 128,
                                    cb * 512:(cb + 1) * 512])
                        nc.vector.tensor_reduce(
                            Y[pt][:, cb * 512:(cb + 1) * 512],
                            stg[:].rearrange("p h x -> p x h"), AX.X, ALU.add)
                    rs = workab.tile([128, 1], F32, tag="rsY", name="rsY")
                    nc.vector.tensor_reduce(rs[:], Y[pt][:], AX.X, ALU.add)
                    nc.vector.reciprocal(rs[:], rs[:])
                    nc.vector.tensor_scalar(rs[:], rs[:], float(RAT[0]), None,
                                            ALU.mult)
                    nc.vector.tensor_scalar(Y[pt][:], Y[pt][:], rs[:], None,
                                            ALU.mult)
                    rep = workab.tile([128, 4096], F32, tag="rep", name="rep")
                    srct = n32[pt // 2]
                    base = (2 * pt) % 4 * 32
                    for ar in range(2):
                        for rp in range(2):
                            nc.sync.dma_start(
                                rep[ar * 64 + rp * 32:ar * 64 + rp * 32 + 32, :],
                                srct[base + ar * 32:base + ar * 32 + 32, :, :]
                                .rearrange("p a b -> p (a b)"))
                    nc.vector.scalar_tensor_tensor(
                        Y[pt][:], rep[:], float(RAT[1]), Y[pt][:],
                        ALU.mult, ALU.add)
                    rep2 = workab.tile([128, 4096], F32, tag="rep", name="rep2")
                    for ar in range(2):
                        a_loc = 2 * pt + ar
                        for rp in range(4):
                            nc.sync.dma_start(
                                rep2[ar * 64 + rp * 16:ar * 64 + rp * 16 + 16, :],
                                n16[a_loc * 16:a_loc * 16 + 16, :, :]
                                .rearrange("p a b -> p (a b)"))
                    nc.vector.scalar_tensor_tensor(
                        Y[pt][:], rep2[:], float(RAT[2]), Y[pt][:],
                        ALU.mult, ALU.add)
                    rep3 = workab.tile([128, 4096], F32, tag="rep", name="rep3")
                    for ar in range(2):
                        a_loc = 2 * pt + ar
                        for rp in range(8):
                            nc.sync.dma_start(
                                rep3[ar * 64 + rp * 8:ar * 64 + rp * 8 + 8, :],
                                n8[(a_loc % 8) * 8:(a_loc % 8) * 8 + 8, :, :]
                                .rearrange("p a b -> p (a b)"))
                    nc.vector.scalar_tensor_tensor(
                        Y[pt][:], rep3[:], float(RAT[3]), Y[pt][:],
                        ALU.mult, ALU.add)

            # ============ phase C: merge0 ============
            with tc.tile_pool(name="xstore", bufs=1) as xstore, \
                 tc.tile_pool(name="workc", bufs=2) as workc:
                # natural-layout fp16 + sy columns
                yh_nat = [xstore.tile([128, 4096], F16, tag=f"yhn{pt}",
                                      name=f"yhn{pt}") for pt in range(4)]
                sycol = [cpool.tile([128, 1], F32, tag=f"syc{mt}",
                                    name=f"syc{mt}") for mt in range(4)]
                for pt in range(4):
                    nc.scalar.activation(yh_nat[pt][:], Y[pt][:], AF.Copy)
                    lnn = workc.tile([128, 4096], F16, tag="lnn", name="lnn",
                                     bufs=1)
                    nc.scalar.activation(lnn[:], yh_nat[pt][:], AF.Ln)
                    nc.vector.tensor_tensor(lnn[:], yh_nat[pt][:], lnn[:],
                                            ALU.mult)
                    nc.vector.tensor_reduce(sycol[pt][:], lnn[:], AX.X,
                                            ALU.add)

                # anchors -> allgather -> X^T, lX^T
                psA_cm = tc.tile_pool(name="psA", bufs=2, space="PSUM")
                psA = psA_cm.__enter__()
                psAs_cm = tc.tile_pool(name="psAs", bufs=1, space="PSUM")
                psAs = psAs_cm.__enter__()
                xloc = workc.tile([32, 4096], F32, tag="xloc", name="xloc",
                                  bufs=1)
                nc.sync.dma_start(xloc[0:16, :], Y[0][65:126:4, :])
                nc.sync.dma_start(xloc[16:32, :], Y[2][65:126:4, :])
                xloch = workc.tile([32, 4096], F16, tag="xloch", name="xloch",
                                   bufs=1)
                nc.scalar.activation(xloch[:], xloc[:], AF.Copy)
                bx_in = dram.tile([32, 4096], F16, name="bx_in")
                bx_out = dram.tile([8, 32, 4096], F16, name="bx_out")
                nc.sync.dma_start(bx_in[:], xloch[:])
                nc.gpsimd.collective_compute(
                    "AllGather", ALU.bypass, replica_groups=RG,
                    ins=[bx_in.opt()], outs=[bx_out.opt()])
                xT = xstore.tile([128, 32, 256], F16, tag="xT", name="xT")
                lxT = xstore.tile([128, 32, 256], F16, tag="lxT", name="lxT")
                bxv = bx_out[:].rearrange("c a p -> (c a) p")
                for nt in range(2):
                    xnat = workc.tile([128, 4096], F16, tag="xnat",
                                      name="xnat", bufs=1)
                    nc.sync.dma_start(xnat[:], bxv[nt * 128:(nt + 1) * 128, :])
                    for ct in range(32):
                        pst = psA.tile([128, 128], F16, tag="tp16",
                                       name="tp16")
                        nc.tensor.transpose(
                            pst[:], xnat[:, ct * 128:(ct + 1) * 128],
                            ident16[:])
                        nc.any.tensor_copy(
                            xT[:, ct, nt * 128:(nt + 1) * 128], pst[:])
                for ct in range(32):
                    nc.scalar.activation(lxT[:, ct, :], xT[:, ct, :], AF.Ln)
                sxP = psAs.tile([1, 256], F32, tag="sx", name="sxP")
                for ct in range(32):
                    prodx = workc.tile([128, 256], F16, tag="prodX",
                                       name="prodX")
                    nc.vector.tensor_tensor(prodx[:], xT[:, ct, :],
                                            lxT[:, ct, :], ALU.mult)
                    nc.tensor.matmul(sxP[:], ones16[:], prodx[:],
                                     start=(ct == 0), stop=(ct == 31))
                sx_sb = workc.tile([1, 256], F32, tag="sxsb", name="sxsb",
                                   bufs=1)
                nc.any.tensor_copy(sx_sb[:], sxP[:])
                sxb = cpool.tile([128, 256], F32, tag="sxb", name="sxb")
                nc.gpsimd.partition_broadcast(sxb[:], sx_sb[:])

                # cross accumulation with rolling transposed ct-tiles
                psC = [psA.tile([128, 256], F32, tag=f"psC{mt}",
                                name=f"psC{mt}", bufs=1) for mt in range(4)]
                for ct in range(32):
                    yhTct = workc.tile([128, 512], F16, tag="yhTct",
                                       name="yhTct")
                    for pt in range(4):
                        pst = psA.tile([128, 128], F16, tag="tp16",
                                       name="tp16b")
                        nc.tensor.transpose(
                            pst[:], yh_nat[pt][:, ct * 128:(ct + 1) * 128],
                            ident16[:])
                        nc.any.tensor_copy(
                            yhTct[:, pt * 128:(pt + 1) * 128], pst[:])
                    lnct = workc.tile([128, 512], F16, tag="lnct", name="lnct")
                    nc.scalar.activation(lnct[:], yhTct[:], AF.Ln)
                    for mt in range(4):
                        nc.tensor.matmul(
                            psC[mt][:], lnct[:, mt * 128:(mt + 1) * 128],
                            xT[:, ct, :], start=(ct == 0), stop=False)
                        nc.tensor.matmul(
                            psC[mt][:], yhTct[:, mt * 128:(mt + 1) * 128],
                            lxT[:, ct, :], start=False, stop=(ct == 31))
                knT = xstore.tile([128, 4, 256], F32, tag="knT", name="knT")
                for mt in range(4):
                    S = workc.tile([128, 256], F32, tag="Ssum", name="Ssum")
                    nc.vector.tensor_scalar(S[:], sxb[:], sycol[mt][:], None,
                                            ALU.add)
                    nc.vector.tensor_tensor(S[:], S[:], psC[mt][:],
                                            ALU.subtract)
                    nc.vector.tensor_scalar(knT[:, mt, :], S[:], float(THR2),
                                            None, ALU.is_lt)

                psAs_cm.__exit__(None, None, None)
                psA_cm.__exit__(None, None, None)
                # new^T partials, counts, collectives
                psB_cm = tc.tile_pool(name="psB", bufs=2, space="PSUM")
                psB = psB_cm.__enter__()
                psBs_cm = tc.tile_pool(name="psBs", bufs=1, space="PSUM")
                psBs = psBs_cm.__enter__()
                cntP = psBs.tile([1, 256], F32, tag="cnt", name="cntP")
                for mt in range(4):
                    nc.tensor.matmul(cntP[:], ones32[:], knT[:, mt, :],
                                     start=(mt == 0), stop=(mt == 3))
                bN_in = dram.tile([4096, 256], F32, name="bN_in")
                for ptile in range(32):
                    psN = psB.tile([128, 256], F32, tag="psN", name="psN")
                    for mt in range(4):
                        nc.tensor.matmul(
                            psN[:], Y[mt][:, ptile * 128:(ptile + 1) * 128],
                            knT[:, mt, :], start=(mt == 0), stop=(mt == 3))
                    npt = workc.tile([128, 256], F32, tag="npt", name="npt")
                    nc.any.tensor_copy(npt[:], psN[:])
                    nc.sync.dma_start(bN_in[ptile * 128:(ptile + 1) * 128, :],
                                      npt[:])
                bC_in = dram.tile([1, 256], F32, name="bC_in")
                bC_out = dram.tile([1, 256], F32, name="bC_out")
                cnt_sb = workc.tile([1, 256], F32, tag="cntsb", name="cntsb",
                                    bufs=1)
                nc.any.tensor_copy(cnt_sb[:], cntP[:])
                nc.sync.dma_start(bC_in[:], cnt_sb[:])
                nc.gpsimd.collective_compute(
                    "AllReduce", ALU.add, replica_groups=RG,
                    ins=[bC_in.opt()], outs=[bC_out.opt()])
                bN_out = dram.tile([512, 256], F32, name="bN_out")
                nc.gpsimd.collective_compute(
                    "ReduceScatter", ALU.add, replica_groups=RG,
                    ins=[bN_in.opt()], outs=[bN_out.opt()])

                # X2^T, lX2^T, sx2 partial, C1 partial, outputs
                cntg = workc.tile([1, 256], F32, tag="cntg", name="cntg",
                                  bufs=1)
                nc.sync.dma_start(cntg[:], bC_out[:])
                nc.vector.reciprocal(cntg[:], cntg[:])
                cb2 = cpool.tile([128, 256], F32, tag="cb2", name="cb2")
                nc.gpsimd.partition_broadcast(cb2[:], cntg[:])
                x2T = [xstore.tile([128, 256], F32, tag=f"x2T{t}",
                                   name=f"x2T{t}") for t in range(4)]
                lx2T = [xstore.tile([128, 256], F32, tag=f"lx2T{t}",
                                    name=f"lx2T{t}") for t in range(4)]
                for t in range(4):
                    nc.sync.dma_start(x2T[t][:],
                                      bN_out[t * 128:(t + 1) * 128, :])
                    nc.vector.tensor_tensor(x2T[t][:], x2T[t][:], cb2[:],
                                            ALU.mult)
                    nc.scalar.activation(lx2T[t][:], x2T[t][:], AF.Ln)
                sx2P = psBs.tile([1, 256], F32, tag="sx2", name="sx2P")
                for t in range(4):
                    prod2 = workc.tile([128, 256], F32, tag="prod2",
                                       name="prod2")
                    nc.vector.tensor_tensor(prod2[:], x2T[t][:], lx2T[t][:],
                                            ALU.mult)
                    nc.tensor.matmul(sx2P[:], ones32[:], prod2[:],
                                     start=(t == 0), stop=(t == 3))
                sx2sb = workc.tile([1, 256], F32, tag="sx2sb", name="sx2sb",
                                   bufs=1)
                nc.any.tensor_copy(sx2sb[:], sx2P[:])
                bS_in = dram.tile([1, 256], F32, name="bS_in")
                bS_out = dram.tile([1, 256], F32, name="bS_out")
                nc.sync.dma_start(bS_in[:], sx2sb[:])
                nc.gpsimd.collective_compute(
                    "AllReduce", ALU.add, replica_groups=RG,
                    ins=[bS_in.opt()], outs=[bS_out.opt()])
                nc.sync.dma_start(o_kl[256:257, :], bS_out[:])
                bC1_in = dram.tile([256, 256], F32, name="bC1_in")
                bC1_out = dram.tile([256, 256], F32, name="bC1_out")
                for it in range(2):
                    psC1 = psB.tile([128, 256], F32, tag="pc1", name="pc1")
                    for kt in range(4):
                        nc.tensor.matmul(
                            psC1[:], x2T[kt][:, it * 128:(it + 1) * 128],
                            lx2T[kt][:], start=(kt == 0), stop=(kt == 3))
                    c1t = workc.tile([128, 256], F32, tag="c1t", name="c1t")
                    nc.any.tensor_copy(c1t[:], psC1[:])
                    nc.sync.dma_start(bC1_in[it * 128:(it + 1) * 128, :],
                                      c1t[:])
                nc.gpsimd.collective_compute(
                    "AllReduce", ALU.add, replica_groups=RG,
                    ins=[bC1_in.opt()], outs=[bC1_out.opt()])
                nc.sync.dma_start(o_kl[0:256, :], bC1_out[:])
                for it in range(2):
                    x2n = workc.tile([128, 512], F32, tag="x2n", name="x2n")
                    for kt in range(4):
                        pst = psB.tile([128, 128], F32, tag="tp32",
                                       name="tp32")
                        nc.tensor.transpose(
                            pst[:], x2T[kt][:, it * 128:(it + 1) * 128],
                            ident32[:])
                        nc.any.tensor_copy(x2n[:, kt * 128:(kt + 1) * 128],
                                           pst[:])
                    nc.sync.dma_start(o_x2[it * 128:(it + 1) * 128, :],
                                      x2n[:])
                psBs_cm.__exit__(None, None, None)
                psB_cm.__exit__(None, None, None)

    nc.finalize()
    return nc, ["w64s", "w32s", "w16s", "w8s"], ["x2slice", "klpack"]



def _build_l2():
    nc, tile, mybir = _build_common()
    from concourse.masks import make_identity
    F32 = mybir.dt.float32
    ALU, AX, AF = mybir.AluOpType, mybir.AxisListType, mybir.ActivationFunctionType

    x2s = nc.dram_tensor("x2s", [256, 512], F32, kind="ExternalInput")
    selT = nc.dram_tensor("sel2T", [256, 256], mybir.dt.uint8,
                          kind="ExternalInput")
    icnt = nc.dram_tensor("icnt2", [256, 1], F32, kind="ExternalInput")
    vrow = nc.dram_tensor("vrow", [1, 256], F32, kind="ExternalInput")
    irow = nc.dram_tensor("irow", [1, 256], F32, kind="ExternalInput")
    iwin = nc.dram_tensor("iwin", [128, 1], mybir.dt.int16,
                          kind="ExternalInput")
    o_n2w = nc.dram_tensor("n2w", [256, L3_W * 64], F32,
                           kind="ExternalOutput")
    o_kl3 = nc.dram_tensor("klpack3", [257, 256], F32, kind="ExternalOutput")
    RG = [list(range(NCORES))]

    with tile.TileContext(nc) as tc:
        with tc.tile_pool(name="sb", bufs=1) as pool, \
             tc.tile_pool(name="work", bufs=2) as work, \
             tc.tile_pool(name="psum", bufs=2, space="PSUM") as psum, \
             tc.tile_pool(name="psumS", bufs=1, space="PSUM") as psumS, \
             tc.tile_pool(name="dram", bufs=1, space="DRAM") as dram:
            ident32 = pool.tile([128, 128], F32, tag="id32", name="id32")
            make_identity(nc, ident32[:])
            ones32 = pool.tile([128, 1], F32, tag="o32", name="o32")
            nc.gpsimd.memset(ones32[:], 1.0)
            iw = pool.tile([128, 1], mybir.dt.int16, tag="iw", name="iw")
            nc.sync.dma_start(iw[:], iwin[:])
            xs = [pool.tile([128, 512], F32, tag=f"xs{t}", name=f"xs{t}") for t in range(2)]
            st = [pool.tile([128, 256], F32, tag=f"st{t}", name=f"st{t}") for t in range(2)]
            stu = [pool.tile([128, 256], mybir.dt.uint8, tag=f"stu{t}",
                             name=f"stu{t}") for t in range(2)]
            for t in range(2):
                nc.sync.dma_start(xs[t][:], x2s[t * 128:(t + 1) * 128, :])
                nc.sync.dma_start(stu[t][:], selT[t * 128:(t + 1) * 128, :])
                nc.any.tensor_copy(st[t][:], stu[t][:])
            cnt = pool.tile([128, 2], F32, tag="cnt", name="cnt")
            nc.sync.dma_start(cnt[:], icnt[:].rearrange("(a p) b -> p (a b)", a=2))
            rc = pool.tile([128, 2], F32, tag="rc", name="rc")
            nc.vector.reciprocal(rc[:], cnt[:])
            vb = pool.tile([128, 256], F32, tag="vb", name="vb")
            ib = pool.tile([128, 256], F32, tag="ib", name="ib")
            vsb = work.tile([1, 256], F32, tag="vsb", name="vsb")
            isb = work.tile([1, 256], F32, tag="isb", name="isb")
            nc.sync.dma_start(vsb[:], vrow[:])
            nc.sync.dma_start(isb[:], irow[:])
            nc.gpsimd.partition_broadcast(vb[:], vsb[:])
            nc.gpsimd.partition_broadcast(ib[:], isb[:])

            new2 = [pool.tile([128, 512], F32, tag=f"n2{t}", name=f"n2{t}") for t in range(2)]
            for mt in range(2):
                ps = psum.tile([128, 512], F32, tag="ps", name="ps")
                for kt in range(2):
                    nc.tensor.matmul(ps[:], st[kt][:, mt * 128:(mt + 1) * 128],
                                     xs[kt][:], start=(kt == 0), stop=(kt == 1))
                nc.vector.tensor_scalar(new2[mt][:], ps[:], rc[:, mt:mt + 1],
                                        None, ALU.mult)
            # allgather new2 -> window rows for L3 (device-chained)
            bG_in = dram.tile([256, 512], F32, name="bG_in")
            bG_out = dram.tile([8, 256, 512], F32, name="bG_out")
            for t in range(2):
                nc.sync.dma_start(bG_in[t * 128:(t + 1) * 128, :], new2[t][:])
            nc.gpsimd.collective_compute(
                "AllGather", ALU.bypass, replica_groups=RG,
                ins=[bG_in.opt()], outs=[bG_out.opt()])
            for rt in range(2):
                n2full = work.tile([128, 64, 64], F32, tag="n2full",
                                   name="n2full", bufs=1)
                n2fv = n2full[:].rearrange("p a b -> p (a b)")
                for k in range(NCORES):
                    nc.sync.dma_start(
                        n2fv[:, k * 512:(k + 1) * 512],
                        bG_out[k, rt * 128:(rt + 1) * 128, :])
                n2wt = work.tile([128, L3_W, 64], F32, tag="n2wt",
                                 name="n2wt", bufs=1)
                nc.gpsimd.ap_gather(n2wt[:], n2full[:], iw[:], channels=128,
                                    num_elems=64, d=64, num_idxs=L3_W)
                nc.sync.dma_start(
                    o_n2w[rt * 128:(rt + 1) * 128, :],
                    n2wt[:].rearrange("p a b -> p (a b)"))
            # transpose new2 -> n2T [4 x [128,256]]
            n2T = [pool.tile([128, 256], F32, tag=f"n2T{t}", name=f"n2T{t}") for t in range(4)]
            for ct in range(4):
                for rt in range(2):
                    pst = psum.tile([128, 128], F32, tag="tp", name="tp")
                    nc.tensor.transpose(
                        pst[:], new2[rt][:, ct * 128:(ct + 1) * 128], ident32[:])
                    nc.any.tensor_copy(n2T[ct][:, rt * 128:(rt + 1) * 128], pst[:])
            # masked = n2T*valid + inv ; ln
            ln2T = [pool.tile([128, 256], F32, tag=f"ln2T{t}", name=f"ln2T{t}") for t in range(4)]
            sx3P = psumS.tile([1, 256], F32, tag="sx3", name="sx3")
            for ct in range(4):
                msk = work.tile([128, 256], F32, tag="msk", name="msk")
                nc.vector.tensor_tensor(msk[:], n2T[ct][:], vb[:], ALU.mult)
                nc.vector.tensor_tensor(msk[:], msk[:], ib[:], ALU.add)
                nc.scalar.activation(ln2T[ct][:], msk[:], AF.Ln)
                prod = work.tile([128, 256], F32, tag="prod", name="prod")
                nc.vector.tensor_tensor(prod[:], n2T[ct][:], ln2T[ct][:], ALU.mult)
                nc.tensor.matmul(sx3P[:], ones32[:], prod[:],
                                 start=(ct == 0), stop=(ct == 3))
            sx3sb = work.tile([1, 256], F32, tag="sx3sb", name="sx3sb")
            nc.any.tensor_copy(sx3sb[:], sx3P[:])
            bS3_in = dram.tile([1, 256], F32, name="bS3_in")
            bS3_out = dram.tile([1, 256], F32, name="bS3_out")
            nc.sync.dma_start(bS3_in[:], sx3sb[:])
            nc.gpsimd.collective_compute(
                "AllReduce", ALU.add, replica_groups=RG,
                ins=[bS3_in.opt()], outs=[bS3_out.opt()])
            nc.sync.dma_start(o_kl3[256:257, :], bS3_out[:])
            bC3_in = dram.tile([256, 256], F32, name="bC3_in")
            bC3_out = dram.tile([256, 256], F32, name="bC3_out")
            for it in range(2):
                psC = psum.tile([128, 256], F32, tag="psC", name="psC")
                for kt in range(4):
                    nc.tensor.matmul(psC[:], n2T[kt][:, it * 128:(it + 1) * 128],
                                     ln2T[kt][:], start=(kt == 0), stop=(kt == 3))
                c3t = work.tile([128, 256], F32, tag="c3t", name="c3t")
                nc.any.tensor_copy(c3t[:], psC[:])
                nc.sync.dma_start(bC3_in[it * 128:(it + 1) * 128, :], c3t[:])
            nc.gpsimd.collective_compute(
                "AllReduce", ALU.add, replica_groups=RG,
                ins=[bC3_in.opt()], outs=[bC3_out.opt()])
            nc.sync.dma_start(o_kl3[0:256, :], bC3_out[:])

    nc.finalize()
    return nc, ["x2s", "sel2T", "icnt2", "vrow", "irow", "iwin"], \
        ["n2w", "klpack3"]


def _build_l3():
    nc, tile, mybir = _build_common()
    from concourse.masks import make_identity
    F32, F16 = mybir.dt.float32, mybir.dt.float16
    I16, U32 = mybir.dt.int16, mybir.dt.uint32
    ALU, AX, AF = mybir.AluOpType, mybir.AxisListType, mybir.ActivationFunctionType

    n2w = nc.dram_tensor("n2w", [256, L3_W * 64], F32, kind="ExternalInput")
    selT = nc.dram_tensor("sel3T", [256, 256], mybir.dt.uint8,
                          kind="ExternalInput")
    icnt = nc.dram_tensor("icnt3", [256, 1], F32, kind="ExternalInput")
    bias = nc.dram_tensor("biasv", [256, 1], F32, kind="ExternalInput")
    idxyc = nc.dram_tensor("idxyc", [128, 2], I16, kind="ExternalInput")
    idxyd = nc.dram_tensor("idxyd", [128, 2], I16, kind="ExternalInput")
    wyr = nc.dram_tensor("wy", [1, 32], F32, kind="ExternalInput")
    idxxc = nc.dram_tensor("idxxc", [128, 512], I16, kind="ExternalInput")
    idxxd = nc.dram_tensor("idxxd", [128, 512], I16, kind="ExternalInput")
    wxr = nc.dram_tensor("wx", [1, 256], F32, kind="ExternalInput")
    o_lab = nc.dram_tensor("lab", [8, 128, 64], F32, kind="ExternalOutput")
    RG = [list(range(NCORES))]

    W = L3_W * 64
    with tile.TileContext(nc) as tc:
        with tc.tile_pool(name="sb", bufs=1) as pool, \
             tc.tile_pool(name="work", bufs=2) as work, \
             tc.tile_pool(name="big", bufs=1) as big, \
             tc.tile_pool(name="psum", bufs=2, space="PSUM") as psum, \
             tc.tile_pool(name="dram", bufs=1, space="DRAM") as dram:
            ident32 = pool.tile([128, 128], F32, tag="id32", name="id32")
            make_identity(nc, ident32[:])
            nw = [pool.tile([128, W], F32, tag=f"nw{t}", name=f"nw{t}") for t in range(2)]
            st = [pool.tile([128, 256], F32, tag=f"st{t}", name=f"st{t}") for t in range(2)]
            stu = [pool.tile([128, 256], mybir.dt.uint8, tag=f"stu{t}",
                             name=f"stu{t}") for t in range(2)]
            for t in range(2):
                nc.sync.dma_start(nw[t][:], n2w[t * 128:(t + 1) * 128, :])
                nc.sync.dma_start(stu[t][:], selT[t * 128:(t + 1) * 128, :])
                nc.any.tensor_copy(st[t][:], stu[t][:])
            cnt = pool.tile([128, 2], F32, tag="cnt", name="cnt")
            nc.sync.dma_start(cnt[:], icnt[:].rearrange("(a p) b -> p (a b)", a=2))
            rc = pool.tile([128, 2], F32, tag="rc", name="rc")
            nc.vector.reciprocal(rc[:], cnt[:])
            bv = pool.tile([128, 2], F32, tag="bv", name="bv")
            nc.sync.dma_start(bv[:], bias[:].rearrange("(a p) b -> p (a b)", a=2))
            iyc = pool.tile([128, 2], I16, tag="iyc", name="iyc")
            iyd = pool.tile([128, 2], I16, tag="iyd", name="iyd")
            ixc = pool.tile([128, 512], I16, tag="ixc", name="ixc")
            ixd = pool.tile([128, 512], I16, tag="ixd", name="ixd")
            for t_, s_ in ((iyc, idxyc), (iyd, idxyd), (ixc, idxxc), (ixd, idxxd)):
                nc.sync.dma_start(t_[:], s_[:])
            wyt = pool.tile([128, 32], F32, tag="wyt", name="wyt")
            wxt = pool.tile([128, 256], F32, tag="wxt", name="wxt")
            wsb = work.tile([1, 32], F32, tag="wsb", name="wsb")
            nc.sync.dma_start(wsb[:], wyr[:])
            nc.gpsimd.partition_broadcast(wyt[:], wsb[:])
            wsb2 = work.tile([1, 256], F32, tag="wsb2", name="wsb2")
            nc.sync.dma_start(wsb2[:], wxr[:])
            nc.gpsimd.partition_broadcast(wxt[:], wsb2[:])

            up = [big.tile([128, 8192, 1], F32, tag=f"up{t}", name=f"up{t}")
                  for t in range(2)]
            for mt in range(2):
                n3 = work.tile([128, W], F32, tag="n3", name="n3")
                for half, (c0, c1) in enumerate(((0, 512), (512, W))):
                    ps = psum.tile([128, c1 - c0], F32, tag=f"ps{half}", name=f"ps{half}")
                    for kt in range(2):
                        nc.tensor.matmul(ps[:],
                                         st[kt][:, mt * 128:(mt + 1) * 128],
                                         nw[kt][:, c0:c1],
                                         start=(kt == 0), stop=(kt == 1))
                    nc.vector.tensor_scalar(n3[:, c0:c1], ps[:],
                                            rc[:, mt:mt + 1], None, ALU.mult)
                nc.vector.tensor_scalar(n3[:], n3[:], bv[:, mt:mt + 1], None,
                                        ALU.add)
                # y-interp via gather: [128,10,64] -> c,d [128,32,64]
                yc = work.tile([128, 32, 64], F32, tag="yc", name="yc")
                yd = work.tile([128, 32, 64], F32, tag="yd", name="yd")
                ydr = work.tile([128, 2048, 1], F32, tag="ydr", name="ydr")
                n3v = n3[:].rearrange("p (y x) -> p y x", y=L3_W)
                nc.gpsimd.ap_gather(yc[:], n3v, iyc[:], channels=128,
                                    num_elems=L3_W, d=64, num_idxs=32)
                nc.gpsimd.ap_gather(yd[:], n3v, iyd[:], channels=128,
                                    num_elems=L3_W, d=64, num_idxs=32)
                yc3 = yc[:]
                yd3 = yd[:]
                ydr3 = ydr[:].rearrange("p (y x) o -> p y (x o)", y=32)
                wy3 = wyt[:, :, None].broadcast_to([128, 32, 64])
                nc.vector.tensor_tensor(ydr3, yd3, yc3, ALU.subtract)
                nc.vector.tensor_tensor(ydr3, ydr3, wy3, ALU.mult)
                nc.vector.tensor_tensor(ydr3, ydr3, yc3, ALU.add)
                # x-interp via gather on [128, 2048, 1] -> [128, 8192]
                xc = big.tile([128, 8192, 1], F32, tag="xc", name="xc")
                xd = up[mt]
                nc.gpsimd.ap_gather(xc[:], ydr[:], ixc[:], channels=128,
                                    num_elems=2048, d=1, num_idxs=8192)
                nc.gpsimd.ap_gather(xd[:], ydr[:], ixd[:], channels=128,
                                    num_elems=2048, d=1, num_idxs=8192)
                xc3 = xc[:].rearrange("p (y j) o -> p y (j o)", y=32)
                xd3 = xd[:].rearrange("p (y j) o -> p y (j o)", y=32)
                wx3 = wxt[:, None, :].broadcast_to([128, 32, 256])
                nc.vector.tensor_tensor(xd3, xd3, xc3, ALU.subtract)
                nc.vector.tensor_tensor(xd3, xd3, wx3, ALU.mult)
                nc.vector.tensor_tensor(xd3, xd3, xc3, ALU.add)
            # transpose + argmax
            lab = pool.tile([128, 64], F32, tag="lab", name="lab")
            upf = [u[:].rearrange("p n o -> p (n o)") for u in up]
            for pt in range(64):
                sc = work.tile([128, 256], F32, tag="sc", name="sc")
                for mt in range(2):
                    pst = psum.tile([128, 128], F32, tag="tp", name="tp")
                    nc.tensor.transpose(
                        pst[:], upf[mt][:, pt * 128:(pt + 1) * 128], ident32[:])
                    nc.any.tensor_copy(sc[:, mt * 128:(mt + 1) * 128], pst[:])
                mx = work.tile([128, 8], F32, tag="mx", name="mx")
                nc.vector.max(mx[:], sc[:])
                mi = work.tile([128, 8], U32, tag="mi", name="mi")
                nc.vector.max_index(mi[:], mx[:], sc[:])
                nc.vector.tensor_copy(lab[:, pt:pt + 1], mi[:, 0:1])
            bL_in = dram.tile([128, 64], F32, name="bL_in")
            bL_out = dram.tile([8, 128, 64], F32, name="bL_out")
            nc.sync.dma_start(bL_in[:], lab[:])
            nc.gpsimd.collective_compute(
                "AllGather", ALU.bypass, replica_groups=RG,
                ins=[bL_in.opt()], outs=[bL_out.opt()])
            nc.sync.dma_start(o_lab[:], bL_out[:])

    nc.finalize()
    return nc, ["n2w", "sel3T", "icnt3", "biasv", "idxyc", "idxyd", "wy",
                "idxxc", "idxxd", "wx"], ["lab"]


# ------------------------------------------------------------------- runner
class _Runner:
    """Cached shard_map-jitted executor for a finalized Bass program
    (modeled on bass2jax.run_bass_via_pjrt, but reusable across calls)."""

    def __init__(self, nc):
        import jax
        import jax.numpy as jnp
        from jax.sharding import Mesh, PartitionSpec, NamedSharding
        from jax.experimental.shard_map import shard_map
        from concourse import bass2jax as b2j
        from concourse import mybir
        b2j.install_neuronx_cc_hook()
        self.jax = jax
        self.np_outs = []
        in_names, out_names, out_avals, zero_outs = [], [], [], []
        partition_name = (nc.partition_id_tensor.name
                          if nc.partition_id_tensor else None)
        for alloc in nc.m.functions[0].allocations:
            if not isinstance(alloc, mybir.MemoryLocationSet):
                continue
            name = alloc.memorylocations[0].name
            if alloc.kind == "ExternalInput":
                if name != partition_name:
                    in_names.append(name)
            elif alloc.kind == "ExternalOutput":
                shape = tuple(alloc.tensor_shape)
                dtype = mybir.dt.np(alloc.dtype)
                out_names.append(name)
                out_avals.append(jax.core.ShapedArray(shape, dtype))
                zero_outs.append(np.zeros(shape, dtype))
        self.in_names, self.out_names = in_names, out_names
        self.zero_outs = zero_outs
        n_params = len(in_names)
        bind_in_names = tuple(in_names + out_names +
                              ([partition_name] if partition_name else []))

        def _body(*args):
            operands = list(args)
            if partition_name is not None:
                operands.append(b2j.partition_id_tensor())
            outs = b2j._bass_exec_p.bind(
                *operands,
                out_avals=tuple(out_avals),
                in_names=bind_in_names,
                out_names=tuple(out_names),
                lowering_input_output_aliases=(),
                sim_require_finite=False,
                sim_require_nnan=False,
                nc=nc,
            )
            return tuple(outs)

        devices = jax.devices()[:NCORES]
        mesh = Mesh(np.asarray(devices), ("core",))
        n_outs = len(out_names)
        in_specs = (PartitionSpec("core"),) * (n_params + n_outs)
        out_specs = (PartitionSpec("core"),) * n_outs
        self.fn = jax.jit(
            shard_map(_body, mesh=mesh, in_specs=in_specs,
                      out_specs=out_specs, check_rep=False),
            donate_argnums=tuple(range(n_params, n_params + n_outs)),
            keep_unused=True)
        self.out_avals = out_avals
        # donated zero output buffers, created on-device (no H2D)
        zsh = NamedSharding(mesh, PartitionSpec("core"))
        zspecs = [((NCORES * z.shape[0], *z.shape[1:]), z.dtype)
                  for z in zero_outs]
        self.zfn = jax.jit(
            lambda: tuple(jnp.zeros(s, d) for s, d in zspecs),
            out_shardings=tuple(zsh for _ in zspecs))
        self.in_sharding = zsh

    def __call__(self, per_core_maps):
        concat_in = [np.concatenate([np.asarray(per_core_maps[c][nm])
                                     for c in range(NCORES)], axis=0)
                     for nm in self.in_names]
        return self.run_concat(concat_in)

    def run_raw(self, concat_in, zeros=None):
        """concat_in: list of [NCORES*s0, ...] arrays (np or device jax).
        Returns tuple of sharded jax output arrays. Pass pre-issued `zeros`
        (from self.zfn()) to overlap zero-buffer creation with earlier work."""
        return self.fn(*concat_in, *(zeros if zeros is not None
                                     else self.zfn()))

    def run_concat(self, concat_in):
        out = self.run_raw(concat_in)
        res = []
        for c in range(NCORES):
            res.append({nm: np.asarray(out[i]).reshape(
                NCORES, *self.out_avals[i].shape)[c]
                for i, nm in enumerate(self.out_names)})
        return res


def _get_runner(name):
    if name not in _PROGS:
        build = {"l1": _build_l1, "l2": _build_l2, "l3": _build_l3}[name]
        nc, ins, outs = build()
        _PROGS[name] = _Runner(nc)
    return _PROGS[name]


# ------------------------------------------------------------------- host math
def _greedy(klmat, valid):
    """Reference greedy loop via 256-bit ints. Returns sel bool [256,256], oc."""
    N = klmat.shape[0]
    adj = (klmat < np.float32(0.9)) & valid[None, :]
    rows = [int.from_bytes(np.packbits(adj[i], bitorder='little').tobytes(),
                           'little') for i in range(N)]
    vbits = int.from_bytes(np.packbits(valid, bitorder='little').tobytes(),
                           'little')
    matched = 0
    sel_rows = []
    for i in range(N):
        if (vbits >> i) & 1 and not (matched >> i) & 1:
            matched |= rows[i]
            sel_rows.append(rows[i])
    sel = np.zeros((N, N), bool)
    for o, r in enumerate(sel_rows):
        sel[o] = np.unpackbits(
            np.frombuffer(r.to_bytes(32, 'little'), np.uint8),
            bitorder='little')[:N]
    return sel, len(sel_rows)


def _klmat_host(sx, C):
    """0.5*(((sx_i+sx_j) - C) - C.T) in f32, matching the reference order."""
    t = (sx[:, None] + sx[None, :]).astype(np.float32)
    t = t - C
    t = t - C.T
    return (np.float32(0.5) * t).astype(np.float32)


def _prep_l1_inputs(w64, w32, w16, w8):
    cat64 = np.empty((64, 512, 4096), np.float32)
    cat32 = np.empty((64, 256, 1024), np.float32)
    cat16 = np.empty((64, 128, 256), np.float32)
    cat8 = np.empty((64, 64, 64), np.float32)
    for k in range(NCORES):
        cat64[8 * k:8 * k + 8] = w64[:, 512 * k:512 * k + 512, :]
        r32 = (8 * k) % 32 * 32
        cat32[8 * k:8 * k + 8] = w32[:, r32:r32 + 256, :]
        r16 = (8 * k) % 16 * 16
        cat16[8 * k:8 * k + 8] = w16[:, r16:r16 + 128, :]
        cat8[8 * k:8 * k + 8] = w8
    return [cat64, cat32, cat16, cat8]


def _segment_one(w64, w32, w16, w8, l1_dev_in=None):
    r1 = _get_runner("l1")
    r2 = _get_runner("l2")
    r3 = _get_runner("l3")
    raw1 = r1.run_raw(l1_dev_in if l1_dev_in is not None
                      else _prep_l1_inputs(w64, w32, w16, w8))
    # issue L2/L3 donated-zero creation now: overlaps L1 execution
    z2 = r2.zfn()
    z3 = r3.zfn()
    x2_dev = raw1[r1.out_names.index("x2slice")]   # [2048,512] sharded
    # klpack is AllReduced on device -> fetch core 0's shard only (1 RTT)
    klp = np.asarray(raw1[r1.out_names.index("klpack")]
                     .addressable_shards[0].data)
    C1, sx2 = klp[0:256], klp[256]
    valid = np.ones(256, bool)
    klmat2 = _klmat_host(sx2, C1)
    klmat2 = np.where(valid[None, :], klmat2, np.float32(np.inf))
    sel2, oc2 = _greedy(klmat2, valid)
    sel2f = sel2.astype(np.float32)
    cnt2 = np.maximum(sel2f.sum(1), 1.0).astype(np.float32)
    valid2 = (np.arange(256) < oc2)

    sel2T = np.ascontiguousarray(sel2f.T.astype(np.uint8))
    vrow = valid2.astype(np.float32)[None, :]
    irow = (1.0 - vrow).astype(np.float32)
    per_name2 = {"x2s": x2_dev,                       # device-chained from L1
                 "sel2T": np.tile(sel2T, (NCORES, 1)),
                 "icnt2": np.tile(cnt2[:, None], (NCORES, 1)),
                 "vrow": np.tile(vrow, (NCORES, 1)),
                 "irow": np.tile(irow, (NCORES, 1)),
                 "iwin": _IWIN_CAT}
    raw2 = r2.run_raw([per_name2[nm] for nm in r2.in_names], zeros=z2)
    n2w_dev = raw2[r2.out_names.index("n2w")]      # [2048,768] sharded
    klp3 = np.asarray(raw2[r2.out_names.index("klpack3")]
                      .addressable_shards[0].data)
    C3, sx3 = klp3[0:256], klp3[256]
    klmat3 = _klmat_host(sx3, C3)
    klmat3 = np.where(valid2[None, :], klmat3, np.float32(np.inf))
    sel3, oc3 = _greedy(klmat3, valid2)
    sel3f = sel3.astype(np.float32)
    cnt3 = np.maximum(sel3f.sum(1), 1.0).astype(np.float32)
    valid3 = (np.arange(256) < oc3)

    sel3T = np.ascontiguousarray(sel3f.T.astype(np.uint8))
    biasv = np.where(valid3, np.float32(0.0), NEG_BIG).astype(np.float32)[:, None]
    per_name3 = {"n2w": n2w_dev,                      # device-chained from L2
                 "sel3T": np.tile(sel3T, (NCORES, 1)),
                 "icnt3": np.tile(cnt3[:, None], (NCORES, 1)),
                 "biasv": np.tile(biasv, (NCORES, 1))}
    per_name3.update(_l3_const_dev())
    raw3 = r3.run_raw([per_name3[nm] for nm in r3.in_names], zeros=z3)
    # lab is AllGathered on device: fetch core 0's shard [8, 128, 64] only
    lab = np.asarray(
        raw3[r3.out_names.index("lab")].addressable_shards[0].data)
    out = np.empty((65536,), np.int32)
    for k in range(NCORES):
        out[8192 * k:8192 * (k + 1)] = lab[k].T.reshape(-1).astype(np.int32)
    return out.reshape(256, 256)


def kernel(**inputs):
    w64 = np.asarray(inputs["weight_64"], np.float32)
    w32 = np.asarray(inputs["weight_32"], np.float32)
    w16 = np.asarray(inputs["weight_16"], np.float32)
    w8 = np.asarray(inputs["weight_8"], np.float32)
    B = w64.shape[0]
    outs = [_segment_one(w64[b], w32[b], w16[b], w8[b]) for b in range(B)]
    return np.stack(outs).astype(np.int32)



# revision 29
# speedup vs baseline: 6.6143x; 6.6143x over previous
"""DiffSeg segmentation head on 8 Trainium2 NeuronCores (Bass/Tile).

Pipeline (per image, B=1):
  L1 (device): multi-scale aggregation of attention maps -> agg [4096,4096] f32
      (row-sharded 512 rows/core), anchor gather + symmetric-KL merge0
      (fp16 log/matmuls like the reference), ReduceScatter -> X2 p-slices,
      partial C1 = X2 @ lX2.T for the first greedy KL matrix.
  host: assemble klmat2, run the sequential greedy selection (bitsets).
  L2 (device): new2 = sel2 @ X2 / cnt, partial C3 for the second greedy.
  host: greedy again.
  L3 (device): new3 = sel3 @ new2 / cnt, 4x bilinear upsample (align_corners),
      invalid-row bias, transpose, per-pixel argmax over anchors.

Self-contained: hardcodes shapes/sharding for inputs
  weight_64 [B,8,4096,4096], weight_32 [B,8,1024,1024],
  weight_16 [B,8,256,256],   weight_8  [B,8,64,64]  (B=1)
"""

import sys
import numpy as np

for _p in ("/opt/trn_rl_repo", "/opt/trn_rl_repo/concourse"):
    if _p not in sys.path:
        sys.path.append(_p)

NCORES = 8
THR2 = np.float32(1.8)          # == 2 * f32(0.9), exact
RAT = [np.float32(64.0 / 120.0), np.float32(32.0 / 120.0),
       np.float32(16.0 / 120.0), np.float32(8.0 / 120.0)]
NEG_BIG = np.float32(-1e38)


def _up_coords(n, r):
    s = np.linspace(0.0, n - 1.0, n * r)
    i0 = np.clip(np.floor(s).astype(np.int64), 0, n - 2)
    w = (s - i0).astype(np.float32)
    return i0.astype(np.int64), w


# ---------------------------------------------------------------- host consts
X032, W32 = _up_coords(32, 2)      # 32 -> 64
X016, W16 = _up_coords(16, 4)
X08, W8 = _up_coords(8, 8)
Y0U, WYU = _up_coords(64, 4)       # 64 -> 256 (final upsample)
X0U, WXU = _up_coords(64, 4)

# L3 per-core y windows
L3_LO = [int(Y0U[32 * k]) for k in range(NCORES)]          # window start row
L3_W = 12  # padded window size (12 = smallest mult-of-4 >= 10)

ANCHOR_PTS = [1 + 4 * i for i in range(16)]


def _wrap_idx(vals):
    """ap_gather index layout: j -> partition j%16, elem j//16; replicated to
    all 8 gpsimd cores (16-partition groups)."""
    n = len(vals)
    cols = (n + 15) // 16
    arr = np.zeros((16, cols), np.int16)
    for j, v in enumerate(vals):
        arr[j % 16, j // 16] = v
    return np.tile(arr, (8, 1))


def _l3_host_consts():
    consts = []
    # x-gather indices (uniform across cores): view [32y*64x]; idx = y*64+x0(j)
    idxx_c = np.array([y * 64 + X0U[j] for y in range(32) for j in range(256)],
                      np.int16)
    idxx_d = np.array([y * 64 + X0U[j] + 1 for y in range(32) for j in range(256)],
                      np.int16)
    wxr = np.broadcast_to(WXU[None, :], (32, 256)).reshape(1, -1).astype(np.float32)
    for k in range(NCORES):
        lo = L3_LO[k]
        y0l = [int(Y0U[32 * k + t]) - lo for t in range(32)]
        consts.append(dict(
            idxyc=_wrap_idx(np.array(y0l, np.int16)),
            idxyd=_wrap_idx(np.array(y0l, np.int16) + 1),
            wy=WYU[32 * k:32 * k + 32][None, :].astype(np.float32),
            idxxc=_wrap_idx(idxx_c),
            idxxd=_wrap_idx(idxx_d),
            wx=WXU[None, :].astype(np.float32),
        ))
    return consts


L3C = _l3_host_consts()
_IWIN_CAT = np.concatenate(
    [_wrap_idx(np.clip(np.arange(L3_LO[k], L3_LO[k] + L3_W), 0, 63)
               .astype(np.int16)) for k in range(NCORES)], axis=0)
_L3_DEV = {}


def _l3_const_dev():
    """Constant L3 inputs, concatenated and staged on device once."""
    if not _L3_DEV:
        import jax
        from jax.sharding import Mesh, PartitionSpec, NamedSharding
        mesh = Mesh(np.asarray(jax.devices()[:NCORES]), ("core",))
        sh = NamedSharding(mesh, PartitionSpec("core"))
        for nm in ("wy", "wx", "idxyc", "idxyd", "idxxc", "idxxd"):
            cat = np.concatenate([L3C[k][nm] for k in range(NCORES)], axis=0)
            _L3_DEV[nm] = jax.device_put(cat, sh)
    return _L3_DEV

# ------------------------------------------------------------------- programs
_PROGS = {}


def _mybir():
    from concourse import mybir
    return mybir


def _build_common():
    import concourse.tile as tile
    from concourse import mybir, bacc
    from concourse.bass_utils import axon_active
    nc = bacc.Bacc("TRN2", target_bir_lowering=False, debug=False,
                   enable_asserts=False, num_devices=NCORES)
    return nc, tile, mybir



def _emit_greedy_consts(nc, tile, mybir, cpool, ident32=None, ones32=None):
    """Constants shared by the on-device greedy passes."""
    from concourse.masks import make_identity
    F32 = mybir.dt.float32
    if ident32 is None:
        ident32 = cpool.tile([128, 128], F32, tag="gid32", name="gid32")
        make_identity(nc, ident32[:])
    if ones32 is None:
        ones32 = cpool.tile([128, 1], F32, tag="go32", name="go32")
        nc.gpsimd.memset(ones32[:], 1.0)
    zrow = cpool.tile([1, 256], F32, tag="gzr", name="gzr")
    nc.vector.memset(zrow[:], 0.0)
    io256f = cpool.tile([128, 256], F32, tag="gio", name="gio")
    nc.gpsimd.iota(io256f[:], pattern=[[1, 256]], base=0,
                   channel_multiplier=0, allow_small_or_imprecise_dtypes=True)
    iocol = [cpool.tile([128, 1], F32, tag=f"gic{t}", name=f"gic{t}")
             for t in range(2)]
    nc.gpsimd.iota(iocol[0][:], pattern=[[1, 1]], base=0,
                   channel_multiplier=1, allow_small_or_imprecise_dtypes=True)
    nc.vector.tensor_scalar(iocol[1][:], iocol[0][:], 128.0, None,
                            mybir.AluOpType.add)
    return dict(ident32=ident32, ones32=ones32, zrow=zrow, io256f=io256f,
                iocol=iocol)


def _emit_greedy(nc, tc, mybir, G, pool, work, psum, psumS, scratch, dram,
                 C_dram, sx_dram, m_init_row, tag):
    """On-device greedy KL merge (reference _merge_greedy selection).

    C_dram: [256,256] DRAM view of AllReduced C = X @ lX.T
    sx_dram: [1,256] DRAM view of AllReduced sx
    m_init_row: SBUF [1,256] f32, 1.0 where column invalid else 0.0
    Returns dict: selT (2 sbuf tiles [128,256] = sel^T), rc (2 cols [128,1]
    reciprocal counts for out rows o=t*128+p), valid_row [1,256], oc [1,1].
    """
    F32 = mybir.dt.float32
    ALU, AX = mybir.AluOpType, mybir.AxisListType
    ident32, io256f, iocol = G["ident32"], G["io256f"], G["iocol"]
    ones32, zrow = G["ones32"], G["zrow"]

    Ct = [work.tile([128, 256], F32, tag=f"{tag}C{t}", name=f"{tag}C{t}",
                    bufs=1) for t in range(2)]
    CT = [work.tile([128, 256], F32, tag=f"{tag}CT{t}", name=f"{tag}CT{t}",
                    bufs=1) for t in range(2)]
    for t in range(2):
        nc.sync.dma_start(Ct[t][:], C_dram[t * 128:(t + 1) * 128, :])
    for t in range(2):
        for u in range(2):
            pst = psum.tile([128, 128], F32, tag="gtp", name=f"{tag}tp")
            nc.tensor.transpose(pst[:], Ct[u][:, t * 128:(t + 1) * 128],
                                ident32[:])
            nc.any.tensor_copy(CT[t][:, u * 128:(u + 1) * 128], pst[:])
    sxr = work.tile([1, 256], F32, tag=f"{tag}sxr", name=f"{tag}sxr")
    nc.sync.dma_start(sxr[:], sx_dram)
    sxb = work.tile([128, 256], F32, tag=f"{tag}sxb", name=f"{tag}sxb")
    nc.gpsimd.partition_broadcast(sxb[:], sxr[:])
    one11 = ones32[0:1, 0:1]

    def row_to_col(row_ap, col_tile, pname):
        pst = psum.tile([128, 128], F32, tag="gtp", name=pname)
        nc.tensor.matmul(pst[:, 0:1], row_ap, one11, start=True, stop=True)
        nc.any.tensor_copy(col_tile[:], pst[:, 0:1])

    sxc = [work.tile([128, 1], F32, tag=f"{tag}sxc{t}", name=f"{tag}sxc{t}")
           for t in range(2)]
    for t in range(2):
        row_to_col(sxr[:, t * 128:(t + 1) * 128], sxc[t], f"{tag}tps")
    # A raw adjacency + A' (diag +1) tiles; A' staged flat on partition 0
    F16 = mybir.dt.float16
    A = [work.tile([128, 256], F32, tag=f"{tag}A{t}", name=f"{tag}A{t}",
                   bufs=1) for t in range(2)]
    Abig = scratch.tile([1, 65536], F16, tag="gAbig", name=f"{tag}Abig")
    dAbig = dram.tile([1, 65536], F16, name=f"{tag}dAbig")
    for t in range(2):
        tmp = work.tile([128, 256], F32, tag=f"{tag}km", name=f"{tag}km")
        nc.vector.tensor_scalar(tmp[:], sxb[:], sxc[t][:], None, ALU.add)
        nc.vector.tensor_tensor(tmp[:], tmp[:], Ct[t][:], ALU.subtract)
        nc.vector.tensor_tensor(tmp[:], tmp[:], CT[t][:], ALU.subtract)
        nc.vector.tensor_scalar(tmp[:], tmp[:], 0.5, None, ALU.mult)
        nc.vector.tensor_scalar(A[t][:], tmp[:], float(np.float32(0.9)),
                                None, ALU.is_lt)
        Aph = work.tile([128, 256], F16, tag=f"{tag}Aph", name=f"{tag}Aph")
        nc.vector.tensor_copy(Aph[:], A[t][:])
        nc.vector.scalar_tensor_tensor(
            Aph[:, t * 128:(t + 1) * 128], ident32[:], 1.0,
            A[t][:, t * 128:(t + 1) * 128], ALU.mult, ALU.add)
        nc.sync.dma_start(
            dAbig[:, t * 32768:(t + 1) * 32768]
            .rearrange("o (p x) -> (o p) x", p=128), Aph[:])
    nc.sync.dma_start(Abig[:], dAbig[:])
    # sequential scan: m=1-valid init; selected rows end at m==2
    m = work.tile([1, 256], F16, tag=f"{tag}m", name=f"{tag}m", bufs=1)
    nc.vector.tensor_copy(m[:], m_init_row)
    act = work.tile([1, 1], F16, tag=f"{tag}act", name=f"{tag}act", bufs=1)
    for i in range(256):
        nc.vector.tensor_scalar(act[:], m[:, i:i + 1], 1.0, None, ALU.is_lt)
        nc.vector.scalar_tensor_tensor(
            m[:], Abig[:, i * 256:(i + 1) * 256], act[:], m[:],
            ALU.mult, ALU.max)
    arow = work.tile([1, 256], F32, tag=f"{tag}ar", name=f"{tag}ar", bufs=1)
    nc.vector.tensor_scalar(arow[:], m[:], 2.0, None, ALU.is_equal)
    oc = pool.tile([1, 1], F32, tag=f"{tag}oc", name=f"{tag}oc")
    nc.vector.tensor_reduce(oc[:], arow[:], AX.X, ALU.add)
    valid_row = pool.tile([1, 256], F32, tag=f"{tag}vn", name=f"{tag}vn")
    nc.vector.tensor_scalar(valid_row[:], io256f[0:1, :], oc[:], None,
                            ALU.is_lt)
    # compaction P^T[i,o] = active_i & (cumsum_i - 1 == o)
    crow = work.tile([1, 256], F32, tag=f"{tag}cr", name=f"{tag}cr")
    nc.vector.tensor_tensor_scan(crow[:], arow[:], zrow[:], 0.0,
                                 ALU.add, ALU.add)
    nc.vector.tensor_scalar(crow[:], crow[:], 1.0, None, ALU.subtract)
    dcol = [work.tile([128, 1], F32, tag=f"{tag}dc{t}", name=f"{tag}dc{t}")
            for t in range(2)]
    acol = [work.tile([128, 1], F32, tag=f"{tag}ac{t}", name=f"{tag}ac{t}")
            for t in range(2)]
    for t in range(2):
        row_to_col(crow[:, t * 128:(t + 1) * 128], dcol[t], f"{tag}tpc")
        row_to_col(arow[:, t * 128:(t + 1) * 128], acol[t], f"{tag}tpa")
    # valid (old) broadcast for column masking of sel rows
    vold = work.tile([1, 256], F32, tag=f"{tag}vo", name=f"{tag}vo")
    nc.vector.tensor_scalar(vold[:], m_init_row, 0.5, None, ALU.is_lt)
    voldb = work.tile([128, 256], F32, tag=f"{tag}vob", name=f"{tag}vob")
    nc.gpsimd.partition_broadcast(voldb[:], vold[:])
    PT = [work.tile([128, 256], F32, tag=f"{tag}PT{t}", name=f"{tag}PT{t}")
          for t in range(2)]
    Am = [work.tile([128, 256], F32, tag=f"{tag}Am{t}", name=f"{tag}Am{t}")
          for t in range(2)]
    for t in range(2):
        nc.vector.tensor_scalar(PT[t][:], io256f[:], dcol[t][:], None,
                                ALU.is_equal)
        nc.vector.tensor_scalar(PT[t][:], PT[t][:], acol[t][:], None,
                                ALU.mult)
        nc.vector.tensor_tensor(Am[t][:], A[t][:], voldb[:], ALU.mult)
    selT = [pool.tile([128, 256], F32, tag=f"{tag}sT{t}", name=f"{tag}sT{t}")
            for t in range(2)]
    for jt in range(2):
        ps = psum.tile([128, 256], F32, tag="gps", name=f"{tag}ps")
        for t in range(2):
            nc.tensor.matmul(ps[:], Am[t][:, jt * 128:(jt + 1) * 128],
                             PT[t][:], start=(t == 0), stop=(t == 1))
        nc.any.tensor_copy(selT[jt][:], ps[:])
    cntP = psumS.tile([1, 256], F32, tag="gcp", name=f"{tag}cp")
    for jt in range(2):
        nc.tensor.matmul(cntP[:], ones32[:], selT[jt][:],
                         start=(jt == 0), stop=(jt == 1))
    cnt = work.tile([1, 256], F32, tag=f"{tag}cn", name=f"{tag}cn")
    nc.vector.tensor_scalar(cnt[:], cntP[:], 1.0, None, ALU.max)
    rc = [pool.tile([128, 1], F32, tag=f"{tag}rc{t}", name=f"{tag}rc{t}")
          for t in range(2)]
    for t in range(2):
        row_to_col(cnt[:, t * 128:(t + 1) * 128], rc[t], f"{tag}tpr")
        nc.vector.reciprocal(rc[t][:], rc[t][:])
    return dict(selT=selT, rc=rc, valid_row=valid_row, oc=oc, arow=arow,
                m=m, Abig=Abig)


def _build_gtest():
    """Standalone test program for the on-device greedy."""
    nc, tile, mybir = _build_common()
    F32 = mybir.dt.float32
    Cin = nc.dram_tensor("Cin", [256, 256], F32, kind="ExternalInput")
    sxin = nc.dram_tensor("sxin", [1, 256], F32, kind="ExternalInput")
    minit = nc.dram_tensor("minit", [1, 256], F32, kind="ExternalInput")
    o_selT = nc.dram_tensor("oselT", [256, 256], F32, kind="ExternalOutput")
    o_aux = nc.dram_tensor("oaux", [8, 256], F32, kind="ExternalOutput")

    with tile.TileContext(nc) as tc:
        with tc.tile_pool(name="cp", bufs=1) as cpool, \
             tc.tile_pool(name="pool", bufs=1) as pool, \
             tc.tile_pool(name="work", bufs=2) as work, \
             tc.tile_pool(name="scr", bufs=1) as scratch, \
             tc.tile_pool(name="ps", bufs=2, space="PSUM") as psum, \
             tc.tile_pool(name="psS", bufs=1, space="PSUM") as psumS, \
             tc.tile_pool(name="dram", bufs=1, space="DRAM") as dram:
            G = _emit_greedy_consts(nc, tile, mybir, cpool)
            mrow = pool.tile([1, 256], F32, tag="mi", name="mi")
            nc.sync.dma_start(mrow[:], minit[:])
            r = _emit_greedy(nc, tc, mybir, G, pool, work, psum, psumS,
                             scratch, dram, Cin[:, :], sxin[:, :], mrow[:],
                             "g")
            for t in range(2):
                nc.sync.dma_start(o_selT[t * 128:(t + 1) * 128, :],
                                  r["selT"][t][:])
            rcrow = work.tile([1, 256], F32, tag="rcr", name="rcr")
            for t in range(2):
                pst = psum.tile([128, 128], F32, tag="gtp", name="gtpo")
                nc.tensor.transpose(pst[0:1, :], r["rc"][t][:],
                                    G["ident32"][:])
                nc.any.tensor_copy(rcrow[:, t * 128:(t + 1) * 128],
                                   pst[0:1, :])
            nc.sync.dma_start(o_aux[0:1, :], rcrow[:])
            nc.sync.dma_start(o_aux[1:2, :], r["valid_row"][:])
            nc.sync.dma_start(o_aux[2:3, :], r["arow"][:])
            ocr = work.tile([1, 256], F32, tag="ocr", name="ocr")
            nc.vector.tensor_scalar(ocr[:], G["zrow"][:], r["oc"][:], None,
                                    mybir.AluOpType.add)
            nc.sync.dma_start(o_aux[3:4, :], ocr[:])
            dbg = work.tile([1, 256], F32, tag="dbg", name="dbg")
            nc.vector.tensor_copy(dbg[:], r["m"][:])
            nc.sync.dma_start(o_aux[4:5, :], dbg[:])
            for k in range(3):
                dbg2 = work.tile([1, 256], F32, tag="dbg2", name=f"dbg2{k}")
                nc.vector.tensor_copy(dbg2[:],
                                      r["Abig"][:, k * 256:(k + 1) * 256])
                nc.sync.dma_start(o_aux[5 + k:6 + k, :], dbg2[:])

    nc.finalize()
    return nc, ["Cin", "sxin", "minit"], ["oselT", "oaux"]


def _build_l1():
    nc, tile, mybir = _build_common()
    from concourse.masks import make_identity
    F32, F16 = mybir.dt.float32, mybir.dt.float16
    ALU, AX, AF = mybir.AluOpType, mybir.AxisListType, mybir.ActivationFunctionType
    RG = [list(range(NCORES))]

    w64 = nc.dram_tensor("w64s", [8, 512, 4096], F32, kind="ExternalInput")
    w32 = nc.dram_tensor("w32s", [8, 256, 1024], F32, kind="ExternalInput")
    w16 = nc.dram_tensor("w16s", [8, 128, 256], F32, kind="ExternalInput")
    w8 = nc.dram_tensor("w8s", [8, 64, 64], F32, kind="ExternalInput")
    o_x2 = nc.dram_tensor("x2slice", [256, 512], F32, kind="ExternalOutput")
    o_kl = nc.dram_tensor("klpack", [257, 256], F32, kind="ExternalOutput")

    with tile.TileContext(nc) as tc:
        with tc.tile_pool(name="consts", bufs=1) as cpool, \
             tc.tile_pool(name="ystore", bufs=1) as ystore, \
             tc.tile_pool(name="dram", bufs=1, space="DRAM") as dram:

            ident16 = cpool.tile([128, 128], F16, tag="id16", name="id16")
            make_identity(nc, ident16[:])
            ident32 = cpool.tile([128, 128], F32, tag="id32", name="id32")
            make_identity(nc, ident32[:])
            ones16 = cpool.tile([128, 1], F16, tag="o16", name="o16")
            nc.gpsimd.memset(ones16[:], 1.0)
            ones32 = cpool.tile([128, 1], F32, tag="o32", name="o32")
            nc.gpsimd.memset(ones32[:], 1.0)

            Y = [ystore.tile([128, 4096], F32, tag=f"Y{pt}", name=f"Y{pt}")
                 for pt in range(4)]

            # ============ phases A+B: aggregation ============
            with tc.tile_pool(name="nmap", bufs=1) as nmap, \
                 tc.tile_pool(name="stage", bufs=2) as stage, \
                 tc.tile_pool(name="workab", bufs=1) as workab:

                def upsample_norm(src_dram, s, P, tidx, out_tile):
                    """head-sum -> bilinear s->64 (x then y) -> normalize."""
                    ssq = s * s
                    m = workab.tile([P, ssq], F32, tag="mA", name="mA")
                    ncb = max(1, ssq // 256)
                    cw = ssq // ncb
                    for cb in range(ncb):
                        stg = stage.tile([P, 8, cw], F32, tag="stg", name="stg")
                        for h in range(8):
                            nc.sync.dma_start(
                                stg[:, h, :],
                                src_dram[h, tidx * P:(tidx + 1) * P,
                                         cb * cw:(cb + 1) * cw])
                        nc.vector.tensor_reduce(
                            m[:, cb * cw:(cb + 1) * cw],
                            stg[:].rearrange("p h x -> p x h"), AX.X, ALU.add)
                    m3 = m[:].rearrange("p (y x) -> p y x", y=s)
                    x0s, wxs = {32: (X032, W32), 16: (X016, W16),
                                8: (X08, W8)}[s]
                    ux = workab.tile([P, s, 64], F32, tag="uxA", name="uxA")
                    tmp = workab.tile([P, s], F32, tag="tmpxA", name="tmpxA")
                    for j in range(64):
                        c = m3[:, :, int(x0s[j])]
                        d = m3[:, :, int(x0s[j]) + 1]
                        eng = nc.vector if j % 2 == 0 else nc.gpsimd
                        eng.tensor_tensor(tmp[:], d, c, ALU.subtract)
                        nc.vector.scalar_tensor_tensor(
                            ux[:, :, j], tmp[:], float(wxs[j]), c,
                            ALU.mult, ALU.add)
                    tmp2 = workab.tile([P, 64], F32, tag="tmpyA", name="tmpyA")
                    for j in range(64):
                        c = ux[:, int(x0s[j]), :]
                        d = ux[:, int(x0s[j]) + 1, :]
                        eng = nc.vector if j % 2 == 0 else nc.gpsimd
                        eng.tensor_tensor(tmp2[:], d, c, ALU.subtract)
                        nc.vector.scalar_tensor_tensor(
                            out_tile[:, j, :], tmp2[:], float(wxs[j]), c,
                            ALU.mult, ALU.add)
                    rs = workab.tile([P, 1], F32, tag="rsA", name="rsA")
                    nc.vector.tensor_reduce(rs[:], out_tile[:], AX.XY, ALU.add)
                    nc.vector.reciprocal(rs[:], rs[:])
                    flat = out_tile[:].rearrange("p a b -> p (a b)")
                    nc.vector.tensor_scalar(flat, flat, rs[:], None, ALU.mult)

                n32 = [nmap.tile([128, 64, 64], F32, tag=f"n32_{t}",
                                 name=f"n32_{t}") for t in range(2)]
                for t in range(2):
                    upsample_norm(w32, 32, 128, t, n32[t])
                n16 = nmap.tile([128, 64, 64], F32, tag="n16", name="n16")
                upsample_norm(w16, 16, 128, 0, n16)
                n8 = nmap.tile([64, 64, 64], F32, tag="n8", name="n8")
                upsample_norm(w8, 8, 64, 0, n8)

                for pt in range(4):
                    for cb in range(8):
                        stg = stage.tile([128, 8, 512], F32, tag="stg",
                                         name="stg64")
                        for h in range(8):
                            nc.sync.dma_start(
                                stg[:, h, :],
                                w64[h, pt * 128:(pt + 1) * 128,
                                    cb * 512:(cb + 1) * 512])
                        nc.vector.tensor_reduce(
                            Y[pt][:, cb * 512:(cb + 1) * 512],
                            stg[:].rearrange("p h x -> p x h"), AX.X, ALU.add)
                    rs = workab.tile([128, 1], F32, tag="rsY", name="rsY")
                    nc.vector.tensor_reduce(rs[:], Y[pt][:], AX.X, ALU.add)
                    nc.vector.reciprocal(rs[:], rs[:])
                    nc.vector.tensor_scalar(rs[:], rs[:], float(RAT[0]), None,
                                            ALU.mult)
                    nc.vector.tensor_scalar(Y[pt][:], Y[pt][:], rs[:], None,
                                            ALU.mult)
                    rep = workab.tile([128, 4096], F32, tag="rep", name="rep")
                    srct = n32[pt // 2]
                    base = (2 * pt) % 4 * 32
                    for ar in range(2):
                        for rp in range(2):
                            nc.sync.dma_start(
                                rep[ar * 64 + rp * 32:ar * 64 + rp * 32 + 32, :],
                                srct[base + ar * 32:base + ar * 32 + 32, :, :]
                                .rearrange("p a b -> p (a b)"))
                    nc.vector.scalar_tensor_tensor(
                        Y[pt][:], rep[:], float(RAT[1]), Y[pt][:],
                        ALU.mult, ALU.add)
                    rep2 = workab.tile([128, 4096], F32, tag="rep", name="rep2")
                    for ar in range(2):
                        a_loc = 2 * pt + ar
                        for rp in range(4):
                            nc.sync.dma_start(
                                rep2[ar * 64 + rp * 16:ar * 64 + rp * 16 + 16, :],
                                n16[a_loc * 16:a_loc * 16 + 16, :, :]
                                .rearrange("p a b -> p (a b)"))
                    nc.vector.scalar_tensor_tensor(
                        Y[pt][:], rep2[:], float(RAT[2]), Y[pt][:],
                        ALU.mult, ALU.add)
                    rep3 = workab.tile([128, 4096], F32, tag="rep", name="rep3")
                    for ar in range(2):
                        a_loc = 2 * pt + ar
                        for rp in range(8):
                            nc.sync.dma_start(
                                rep3[ar * 64 + rp * 8:ar * 64 + rp * 8 + 8, :],
                                n8[(a_loc % 8) * 8:(a_loc % 8) * 8 + 8, :, :]
                                .rearrange("p a b -> p (a b)"))
                    nc.vector.scalar_tensor_tensor(
                        Y[pt][:], rep3[:], float(RAT[3]), Y[pt][:],
                        ALU.mult, ALU.add)

            # ============ phase C: merge0 ============
            with tc.tile_pool(name="xstore", bufs=1) as xstore, \
                 tc.tile_pool(name="workc", bufs=2) as workc:
                # natural-layout fp16 + sy columns
                yh_nat = [xstore.tile([128, 4096], F16, tag=f"yhn{pt}",
                                      name=f"yhn{pt}") for pt in range(4)]
                sycol = [cpool.tile([128, 1], F32, tag=f"syc{mt}",
                                    name=f"syc{mt}") for mt in range(4)]
                for pt in range(4):
                    nc.scalar.activation(yh_nat[pt][:], Y[pt][:], AF.Copy)
                    lnn = workc.tile([128, 4096], F16, tag="lnn", name="lnn",
                                     bufs=1)
                    nc.scalar.activation(lnn[:], yh_nat[pt][:], AF.Ln)
                    nc.vector.tensor_tensor(lnn[:], yh_nat[pt][:], lnn[:],
                                            ALU.mult)
                    nc.vector.tensor_reduce(sycol[pt][:], lnn[:], AX.X,
                                            ALU.add)

                # anchors -> allgather -> X^T, lX^T
                psA_cm = tc.tile_pool(name="psA", bufs=2, space="PSUM")
                psA = psA_cm.__enter__()
                psAs_cm = tc.tile_pool(name="psAs", bufs=1, space="PSUM")
                psAs = psAs_cm.__enter__()
                xloc = workc.tile([32, 4096], F32, tag="xloc", name="xloc",
                                  bufs=1)
                nc.sync.dma_start(xloc[0:16, :], Y[0][65:126:4, :])
                nc.sync.dma_start(xloc[16:32, :], Y[2][65:126:4, :])
                xloch = workc.tile([32, 4096], F16, tag="xloch", name="xloch",
                                   bufs=1)
                nc.scalar.activation(xloch[:], xloc[:], AF.Copy)
                bx_in = dram.tile([32, 4096], F16, name="bx_in")
                bx_out = dram.tile([8, 32, 4096], F16, name="bx_out")
                nc.sync.dma_start(bx_in[:], xloch[:])
                nc.gpsimd.collective_compute(
                    "AllGather", ALU.bypass, replica_groups=RG,
                    ins=[bx_in.opt()], outs=[bx_out.opt()])
                xT = xstore.tile([128, 32, 256], F16, tag="xT", name="xT")
                lxT = xstore.tile([128, 32, 256], F16, tag="lxT", name="lxT")
                bxv = bx_out[:].rearrange("c a p -> (c a) p")
                for nt in range(2):
                    xnat = workc.tile([128, 4096], F16, tag="xnat",
                                      name="xnat", bufs=1)
                    nc.sync.dma_start(xnat[:], bxv[nt * 128:(nt + 1) * 128, :])
                    for ct in range(32):
                        pst = psA.tile([128, 128], F16, tag="tp16",
                                       name="tp16")
                        nc.tensor.transpose(
                            pst[:], xnat[:, ct * 128:(ct + 1) * 128],
                            ident16[:])
                        nc.any.tensor_copy(
                            xT[:, ct, nt * 128:(nt + 1) * 128], pst[:])
                for ct in range(32):
                    nc.scalar.activation(lxT[:, ct, :], xT[:, ct, :], AF.Ln)
                sxP = psAs.tile([1, 256], F32, tag="sx", name="sxP")
                for ct in range(32):
                    prodx = workc.tile([128, 256], F16, tag="prodX",
                                       name="prodX")
                    nc.vector.tensor_tensor(prodx[:], xT[:, ct, :],
                                            lxT[:, ct, :], ALU.mult)
                    nc.tensor.matmul(sxP[:], ones16[:], prodx[:],
                                     start=(ct == 0), stop=(ct == 31))
                sx_sb = workc.tile([1, 256], F32, tag="sxsb", name="sxsb",
                                   bufs=1)
                nc.any.tensor_copy(sx_sb[:], sxP[:])
                sxb = cpool.tile([128, 256], F32, tag="sxb", name="sxb")
                nc.gpsimd.partition_broadcast(sxb[:], sx_sb[:])

                # cross accumulation with rolling transposed ct-tiles
                psC = [psA.tile([128, 256], F32, tag=f"psC{mt}",
                                name=f"psC{mt}", bufs=1) for mt in range(4)]
                for ct in range(32):
                    yhTct = workc.tile([128, 512], F16, tag="yhTct",
                                       name="yhTct")
                    for pt in range(4):
                        pst = psA.tile([128, 128], F16, tag="tp16",
                                       name="tp16b")
                        nc.tensor.transpose(
                            pst[:], yh_nat[pt][:, ct * 128:(ct + 1) * 128],
                            ident16[:])
                        nc.any.tensor_copy(
                            yhTct[:, pt * 128:(pt + 1) * 128], pst[:])
                    lnct = workc.tile([128, 512], F16, tag="lnct", name="lnct")
                    nc.scalar.activation(lnct[:], yhTct[:], AF.Ln)
                    for mt in range(4):
                        nc.tensor.matmul(
                            psC[mt][:], lnct[:, mt * 128:(mt + 1) * 128],
                            xT[:, ct, :], start=(ct == 0), stop=False)
                        nc.tensor.matmul(
                            psC[mt][:], yhTct[:, mt * 128:(mt + 1) * 128],
                            lxT[:, ct, :], start=False, stop=(ct == 31))
                knT = xstore.tile([128, 4, 256], F32, tag="knT", name="knT")
                for mt in range(4):
                    S = workc.tile([128, 256], F32, tag="Ssum", name="Ssum")
                    nc.vector.tensor_scalar(S[:], sxb[:], sycol[mt][:], None,
                                            ALU.add)
                    nc.vector.tensor_tensor(S[:], S[:], psC[mt][:],
                                            ALU.subtract)
                    nc.vector.tensor_scalar(knT[:, mt, :], S[:], float(THR2),
                                            None, ALU.is_lt)

                psAs_cm.__exit__(None, None, None)
                psA_cm.__exit__(None, None, None)
                # new^T partials, counts, collectives
                psB_cm = tc.tile_pool(name="psB", bufs=2, space="PSUM")
                psB = psB_cm.__enter__()
                psBs_cm = tc.tile_pool(name="psBs", bufs=1, space="PSUM")
                psBs = psBs_cm.__enter__()
                cntP = psBs.tile([1, 256], F32, tag="cnt", name="cntP")
                for mt in range(4):
                    nc.tensor.matmul(cntP[:], ones32[:], knT[:, mt, :],
                                     start=(mt == 0), stop=(mt == 3))
                bN_in = dram.tile([4096, 256], F32, name="bN_in")
                for ptile in range(32):
                    psN = psB.tile([128, 256], F32, tag="psN", name="psN")
                    for mt in range(4):
                        nc.tensor.matmul(
                            psN[:], Y[mt][:, ptile * 128:(ptile + 1) * 128],
                            knT[:, mt, :], start=(mt == 0), stop=(mt == 3))
                    npt = workc.tile([128, 256], F32, tag="npt", name="npt")
                    nc.any.tensor_copy(npt[:], psN[:])
                    nc.sync.dma_start(bN_in[ptile * 128:(ptile + 1) * 128, :],
                                      npt[:])
                bC_in = dram.tile([1, 256], F32, name="bC_in")
                bC_out = dram.tile([1, 256], F32, name="bC_out")
                cnt_sb = workc.tile([1, 256], F32, tag="cntsb", name="cntsb",
                                    bufs=1)
                nc.any.tensor_copy(cnt_sb[:], cntP[:])
                nc.sync.dma_start(bC_in[:], cnt_sb[:])
                nc.gpsimd.collective_compute(
                    "AllReduce", ALU.add, replica_groups=RG,
                    ins=[bC_in.opt()], outs=[bC_out.opt()])
                bN_out = dram.tile([512, 256], F32, name="bN_out")
                nc.gpsimd.collective_compute(
                    "ReduceScatter", ALU.add, replica_groups=RG,
                    ins=[bN_in.opt()], outs=[bN_out.opt()])

                # X2^T, lX2^T, sx2 partial, C1 partial, outputs
                cntg = workc.tile([1, 256], F32, tag="cntg", name="cntg",
                                  bufs=1)
                nc.sync.dma_start(cntg[:], bC_out[:])
                nc.vector.reciprocal(cntg[:], cntg[:])
                cb2 = cpool.tile([128, 256], F32, tag="cb2", name="cb2")
                nc.gpsimd.partition_broadcast(cb2[:], cntg[:])
                x2T = [xstore.tile([128, 256], F32, tag=f"x2T{t}",
                                   name=f"x2T{t}") for t in range(4)]
                lx2T = [xstore.tile([128, 256], F32, tag=f"lx2T{t}",
                                    name=f"lx2T{t}") for t in range(4)]
                for t in range(4):
                    nc.sync.dma_start(x2T[t][:],
                                      bN_out[t * 128:(t + 1) * 128, :])
                    nc.vector.tensor_tensor(x2T[t][:], x2T[t][:], cb2[:],
                                            ALU.mult)
                    nc.scalar.activation(lx2T[t][:], x2T[t][:], AF.Ln)
                sx2P = psBs.tile([1, 256], F32, tag="sx2", name="sx2P")
                for t in range(4):
                    prod2 = workc.tile([128, 256], F32, tag="prod2",
                                       name="prod2")
                    nc.vector.tensor_tensor(prod2[:], x2T[t][:], lx2T[t][:],
                                            ALU.mult)
                    nc.tensor.matmul(sx2P[:], ones32[:], prod2[:],
                                     start=(t == 0), stop=(t == 3))
                sx2sb = workc.tile([1, 256], F32, tag="sx2sb", name="sx2sb",
                                   bufs=1)
                nc.any.tensor_copy(sx2sb[:], sx2P[:])
                bS_in = dram.tile([1, 256], F32, name="bS_in")
                bS_out = dram.tile([1, 256], F32, name="bS_out")
                nc.sync.dma_start(bS_in[:], sx2sb[:])
                nc.gpsimd.collective_compute(
                    "AllReduce", ALU.add, replica_groups=RG,
                    ins=[bS_in.opt()], outs=[bS_out.opt()])
                nc.sync.dma_start(o_kl[256:257, :], bS_out[:])
                bC1_in = dram.tile([256, 256], F32, name="bC1_in")
                bC1_out = dram.tile([256, 256], F32, name="bC1_out")
                for it in range(2):
                    psC1 = psB.tile([128, 256], F32, tag="pc1", name="pc1")
                    for kt in range(4):
                        nc.tensor.matmul(
                            psC1[:], x2T[kt][:, it * 128:(it + 1) * 128],
                            lx2T[kt][:], start=(kt == 0), stop=(kt == 3))
                    c1t = workc.tile([128, 256], F32, tag="c1t", name="c1t")
                    nc.any.tensor_copy(c1t[:], psC1[:])
                    nc.sync.dma_start(bC1_in[it * 128:(it + 1) * 128, :],
                                      c1t[:])
                nc.gpsimd.collective_compute(
                    "AllReduce", ALU.add, replica_groups=RG,
                    ins=[bC1_in.opt()], outs=[bC1_out.opt()])
                nc.sync.dma_start(o_kl[0:256, :], bC1_out[:])
                for it in range(2):
                    x2n = workc.tile([128, 512], F32, tag="x2n", name="x2n")
                    for kt in range(4):
                        pst = psB.tile([128, 128], F32, tag="tp32",
                                       name="tp32")
                        nc.tensor.transpose(
                            pst[:], x2T[kt][:, it * 128:(it + 1) * 128],
                            ident32[:])
                        nc.any.tensor_copy(x2n[:, kt * 128:(kt + 1) * 128],
                                           pst[:])
                    nc.sync.dma_start(o_x2[it * 128:(it + 1) * 128, :],
                                      x2n[:])
                psBs_cm.__exit__(None, None, None)
                psB_cm.__exit__(None, None, None)

    nc.finalize()
    return nc, ["w64s", "w32s", "w16s", "w8s"], ["x2slice", "klpack"]



def _build_l2():
    nc, tile, mybir = _build_common()
    from concourse.masks import make_identity
    F32 = mybir.dt.float32
    ALU, AX, AF = mybir.AluOpType, mybir.AxisListType, mybir.ActivationFunctionType

    x2s = nc.dram_tensor("x2s", [256, 512], F32, kind="ExternalInput")
    selT = nc.dram_tensor("sel2T", [256, 256], mybir.dt.uint8,
                          kind="ExternalInput")
    icnt = nc.dram_tensor("icnt2", [256, 1], F32, kind="ExternalInput")
    vrow = nc.dram_tensor("vrow", [1, 256], F32, kind="ExternalInput")
    irow = nc.dram_tensor("irow", [1, 256], F32, kind="ExternalInput")
    iwin = nc.dram_tensor("iwin", [128, 1], mybir.dt.int16,
                          kind="ExternalInput")
    o_n2w = nc.dram_tensor("n2w", [256, L3_W * 64], F32,
                           kind="ExternalOutput")
    o_kl3 = nc.dram_tensor("klpack3", [257, 256], F32, kind="ExternalOutput")
    RG = [list(range(NCORES))]

    with tile.TileContext(nc) as tc:
        with tc.tile_pool(name="sb", bufs=1) as pool, \
             tc.tile_pool(name="work", bufs=2) as work, \
             tc.tile_pool(name="psum", bufs=2, space="PSUM") as psum, \
             tc.tile_pool(name="psumS", bufs=1, space="PSUM") as psumS, \
             tc.tile_pool(name="dram", bufs=1, space="DRAM") as dram:
            ident32 = pool.tile([128, 128], F32, tag="id32", name="id32")
            make_identity(nc, ident32[:])
            ones32 = pool.tile([128, 1], F32, tag="o32", name="o32")
            nc.gpsimd.memset(ones32[:], 1.0)
            iw = pool.tile([128, 1], mybir.dt.int16, tag="iw", name="iw")
            nc.sync.dma_start(iw[:], iwin[:])
            xs = [pool.tile([128, 512], F32, tag=f"xs{t}", name=f"xs{t}") for t in range(2)]
            st = [pool.tile([128, 256], F32, tag=f"st{t}", name=f"st{t}") for t in range(2)]
            stu = [pool.tile([128, 256], mybir.dt.uint8, tag=f"stu{t}",
                             name=f"stu{t}") for t in range(2)]
            for t in range(2):
                nc.sync.dma_start(xs[t][:], x2s[t * 128:(t + 1) * 128, :])
                nc.sync.dma_start(stu[t][:], selT[t * 128:(t + 1) * 128, :])
                nc.any.tensor_copy(st[t][:], stu[t][:])
            cnt = pool.tile([128, 2], F32, tag="cnt", name="cnt")
            nc.sync.dma_start(cnt[:], icnt[:].rearrange("(a p) b -> p (a b)", a=2))
            rc = pool.tile([128, 2], F32, tag="rc", name="rc")
            nc.vector.reciprocal(rc[:], cnt[:])
            vb = pool.tile([128, 256], F32, tag="vb", name="vb")
            ib = pool.tile([128, 256], F32, tag="ib", name="ib")
            vsb = work.tile([1, 256], F32, tag="vsb", name="vsb")
            isb = work.tile([1, 256], F32, tag="isb", name="isb")
            nc.sync.dma_start(vsb[:], vrow[:])
            nc.sync.dma_start(isb[:], irow[:])
            nc.gpsimd.partition_broadcast(vb[:], vsb[:])
            nc.gpsimd.partition_broadcast(ib[:], isb[:])

            new2 = [pool.tile([128, 512], F32, tag=f"n2{t}", name=f"n2{t}") for t in range(2)]
            for mt in range(2):
                ps = psum.tile([128, 512], F32, tag="ps", name="ps")
                for kt in range(2):
                    nc.tensor.matmul(ps[:], st[kt][:, mt * 128:(mt + 1) * 128],
                                     xs[kt][:], start=(kt == 0), stop=(kt == 1))
                nc.vector.tensor_scalar(new2[mt][:], ps[:], rc[:, mt:mt + 1],
                                        None, ALU.mult)
            # allgather new2 -> window rows for L3 (device-chained)
            bG_in = dram.tile([256, 512], F32, name="bG_in")
            bG_out = dram.tile([8, 256, 512], F32, name="bG_out")
            for t in range(2):
                nc.sync.dma_start(bG_in[t * 128:(t + 1) * 128, :], new2[t][:])
            nc.gpsimd.collective_compute(
                "AllGather", ALU.bypass, replica_groups=RG,
                ins=[bG_in.opt()], outs=[bG_out.opt()])
            for rt in range(2):
                n2full = work.tile([128, 64, 64], F32, tag="n2full",
                                   name="n2full", bufs=1)
                n2fv = n2full[:].rearrange("p a b -> p (a b)")
                for k in range(NCORES):
                    nc.sync.dma_start(
                        n2fv[:, k * 512:(k + 1) * 512],
                        bG_out[k, rt * 128:(rt + 1) * 128, :])
                n2wt = work.tile([128, L3_W, 64], F32, tag="n2wt",
                                 name="n2wt", bufs=1)
                nc.gpsimd.ap_gather(n2wt[:], n2full[:], iw[:], channels=128,
                                    num_elems=64, d=64, num_idxs=L3_W)
                nc.sync.dma_start(
                    o_n2w[rt * 128:(rt + 1) * 128, :],
                    n2wt[:].rearrange("p a b -> p (a b)"))
            # transpose new2 -> n2T [4 x [128,256]]
            n2T = [pool.tile([128, 256], F32, tag=f"n2T{t}", name=f"n2T{t}") for t in range(4)]
            for ct in range(4):
                for rt in range(2):
                    pst = psum.tile([128, 128], F32, tag="tp", name="tp")
                    nc.tensor.transpose(
                        pst[:], new2[rt][:, ct * 128:(ct + 1) * 128], ident32[:])
                    nc.any.tensor_copy(n2T[ct][:, rt * 128:(rt + 1) * 128], pst[:])
            # masked = n2T*valid + inv ; ln
            ln2T = [pool.tile([128, 256], F32, tag=f"ln2T{t}", name=f"ln2T{t}") for t in range(4)]
            sx3P = psumS.tile([1, 256], F32, tag="sx3", name="sx3")
            for ct in range(4):
                msk = work.tile([128, 256], F32, tag="msk", name="msk")
                nc.vector.tensor_tensor(msk[:], n2T[ct][:], vb[:], ALU.mult)
                nc.vector.tensor_tensor(msk[:], msk[:], ib[:], ALU.add)
                nc.scalar.activation(ln2T[ct][:], msk[:], AF.Ln)
                prod = work.tile([128, 256], F32, tag="prod", name="prod")
                nc.vector.tensor_tensor(prod[:], n2T[ct][:], ln2T[ct][:], ALU.mult)
                nc.tensor.matmul(sx3P[:], ones32[:], prod[:],
                                 start=(ct == 0), stop=(ct == 3))
            sx3sb = work.tile([1, 256], F32, tag="sx3sb", name="sx3sb")
            nc.any.tensor_copy(sx3sb[:], sx3P[:])
            bS3_in = dram.tile([1, 256], F32, name="bS3_in")
            bS3_out = dram.tile([1, 256], F32, name="bS3_out")
            nc.sync.dma_start(bS3_in[:], sx3sb[:])
            nc.gpsimd.collective_compute(
                "AllReduce", ALU.add, replica_groups=RG,
                ins=[bS3_in.opt()], outs=[bS3_out.opt()])
            nc.sync.dma_start(o_kl3[256:257, :], bS3_out[:])
            bC3_in = dram.tile([256, 256], F32, name="bC3_in")
            bC3_out = dram.tile([256, 256], F32, name="bC3_out")
            for it in range(2):
                psC = psum.tile([128, 256], F32, tag="psC", name="psC")
                for kt in range(4):
                    nc.tensor.matmul(psC[:], n2T[kt][:, it * 128:(it + 1) * 128],
                                     ln2T[kt][:], start=(kt == 0), stop=(kt == 3))
                c3t = work.tile([128, 256], F32, tag="c3t", name="c3t")
                nc.any.tensor_copy(c3t[:], psC[:])
                nc.sync.dma_start(bC3_in[it * 128:(it + 1) * 128, :], c3t[:])
            nc.gpsimd.collective_compute(
                "AllReduce", ALU.add, replica_groups=RG,
                ins=[bC3_in.opt()], outs=[bC3_out.opt()])
            nc.sync.dma_start(o_kl3[0:256, :], bC3_out[:])

    nc.finalize()
    return nc, ["x2s", "sel2T", "icnt2", "vrow", "irow", "iwin"], \
        ["n2w", "klpack3"]


def _build_l3():
    nc, tile, mybir = _build_common()
    from concourse.masks import make_identity
    F32, F16 = mybir.dt.float32, mybir.dt.float16
    I16, U32 = mybir.dt.int16, mybir.dt.uint32
    ALU, AX, AF = mybir.AluOpType, mybir.AxisListType, mybir.ActivationFunctionType

    n2w = nc.dram_tensor("n2w", [256, L3_W * 64], F32, kind="ExternalInput")
    selT = nc.dram_tensor("sel3T", [256, 256], mybir.dt.uint8,
                          kind="ExternalInput")
    icnt = nc.dram_tensor("icnt3", [256, 1], F32, kind="ExternalInput")
    bias = nc.dram_tensor("biasv", [256, 1], F32, kind="ExternalInput")
    idxyc = nc.dram_tensor("idxyc", [128, 2], I16, kind="ExternalInput")
    idxyd = nc.dram_tensor("idxyd", [128, 2], I16, kind="ExternalInput")
    wyr = nc.dram_tensor("wy", [1, 32], F32, kind="ExternalInput")
    idxxc = nc.dram_tensor("idxxc", [128, 512], I16, kind="ExternalInput")
    idxxd = nc.dram_tensor("idxxd", [128, 512], I16, kind="ExternalInput")
    wxr = nc.dram_tensor("wx", [1, 256], F32, kind="ExternalInput")
    o_lab = nc.dram_tensor("lab", [8, 128, 64], F32, kind="ExternalOutput")
    RG = [list(range(NCORES))]

    W = L3_W * 64
    with tile.TileContext(nc) as tc:
        with tc.tile_pool(name="sb", bufs=1) as pool, \
             tc.tile_pool(name="work", bufs=2) as work, \
             tc.tile_pool(name="big", bufs=1) as big, \
             tc.tile_pool(name="psum", bufs=2, space="PSUM") as psum, \
             tc.tile_pool(name="dram", bufs=1, space="DRAM") as dram:
            ident32 = pool.tile([128, 128], F32, tag="id32", name="id32")
            make_identity(nc, ident32[:])
            nw = [pool.tile([128, W], F32, tag=f"nw{t}", name=f"nw{t}") for t in range(2)]
            st = [pool.tile([128, 256], F32, tag=f"st{t}", name=f"st{t}") for t in range(2)]
            stu = [pool.tile([128, 256], mybir.dt.uint8, tag=f"stu{t}",
                             name=f"stu{t}") for t in range(2)]
            for t in range(2):
                nc.sync.dma_start(nw[t][:], n2w[t * 128:(t + 1) * 128, :])
                nc.sync.dma_start(stu[t][:], selT[t * 128:(t + 1) * 128, :])
                nc.any.tensor_copy(st[t][:], stu[t][:])
            cnt = pool.tile([128, 2], F32, tag="cnt", name="cnt")
            nc.sync.dma_start(cnt[:], icnt[:].rearrange("(a p) b -> p (a b)", a=2))
            rc = pool.tile([128, 2], F32, tag="rc", name="rc")
            nc.vector.reciprocal(rc[:], cnt[:])
            bv = pool.tile([128, 2], F32, tag="bv", name="bv")
            nc.sync.dma_start(bv[:], bias[:].rearrange("(a p) b -> p (a b)", a=2))
            iyc = pool.tile([128, 2], I16, tag="iyc", name="iyc")
            iyd = pool.tile([128, 2], I16, tag="iyd", name="iyd")
            ixc = pool.tile([128, 512], I16, tag="ixc", name="ixc")
            ixd = pool.tile([128, 512], I16, tag="ixd", name="ixd")
            for t_, s_ in ((iyc, idxyc), (iyd, idxyd), (ixc, idxxc), (ixd, idxxd)):
                nc.sync.dma_start(t_[:], s_[:])
            wyt = pool.tile([128, 32], F32, tag="wyt", name="wyt")
            wxt = pool.tile([128, 256], F32, tag="wxt", name="wxt")
            wsb = work.tile([1, 32], F32, tag="wsb", name="wsb")
            nc.sync.dma_start(wsb[:], wyr[:])
            nc.gpsimd.partition_broadcast(wyt[:], wsb[:])
            wsb2 = work.tile([1, 256], F32, tag="wsb2", name="wsb2")
            nc.sync.dma_start(wsb2[:], wxr[:])
            nc.gpsimd.partition_broadcast(wxt[:], wsb2[:])

            up = [big.tile([128, 8192, 1], F32, tag=f"up{t}", name=f"up{t}")
                  for t in range(2)]
            for mt in range(2):
                n3 = work.tile([128, W], F32, tag="n3", name="n3")
                for half, (c0, c1) in enumerate(((0, 512), (512, W))):
                    ps = psum.tile([128, c1 - c0], F32, tag=f"ps{half}", name=f"ps{half}")
                    for kt in range(2):
                        nc.tensor.matmul(ps[:],
                                         st[kt][:, mt * 128:(mt + 1) * 128],
                                         nw[kt][:, c0:c1],
                                         start=(kt == 0), stop=(kt == 1))
                    nc.vector.tensor_scalar(n3[:, c0:c1], ps[:],
                                            rc[:, mt:mt + 1], None, ALU.mult)
                nc.vector.tensor_scalar(n3[:], n3[:], bv[:, mt:mt + 1], None,
                                        ALU.add)
                # y-interp via gather: [128,10,64] -> c,d [128,32,64]
                yc = work.tile([128, 32, 64], F32, tag="yc", name="yc")
                yd = work.tile([128, 32, 64], F32, tag="yd", name="yd")
                ydr = work.tile([128, 2048, 1], F32, tag="ydr", name="ydr")
                n3v = n3[:].rearrange("p (y x) -> p y x", y=L3_W)
                nc.gpsimd.ap_gather(yc[:], n3v, iyc[:], channels=128,
                                    num_elems=L3_W, d=64, num_idxs=32)
                nc.gpsimd.ap_gather(yd[:], n3v, iyd[:], channels=128,
                                    num_elems=L3_W, d=64, num_idxs=32)
                yc3 = yc[:]
                yd3 = yd[:]
                ydr3 = ydr[:].rearrange("p (y x) o -> p y (x o)", y=32)
                wy3 = wyt[:, :, None].broadcast_to([128, 32, 64])
                nc.vector.tensor_tensor(ydr3, yd3, yc3, ALU.subtract)
                nc.vector.tensor_tensor(ydr3, ydr3, wy3, ALU.mult)
                nc.vector.tensor_tensor(ydr3, ydr3, yc3, ALU.add)
                # x-interp via gather on [128, 2048, 1] -> [128, 8192]
                xc = big.tile([128, 8192, 1], F32, tag="xc", name="xc")
                xd = up[mt]
                nc.gpsimd.ap_gather(xc[:], ydr[:], ixc[:], channels=128,
                                    num_elems=2048, d=1, num_idxs=8192)
                nc.gpsimd.ap_gather(xd[:], ydr[:], ixd[:], channels=128,
                                    num_elems=2048, d=1, num_idxs=8192)
                xc3 = xc[:].rearrange("p (y j) o -> p y (j o)", y=32)
                xd3 = xd[:].rearrange("p (y j) o -> p y (j o)", y=32)
                wx3 = wxt[:, None, :].broadcast_to([128, 32, 256])
                nc.vector.tensor_tensor(xd3, xd3, xc3, ALU.subtract)
                nc.vector.tensor_tensor(xd3, xd3, wx3, ALU.mult)
                nc.vector.tensor_tensor(xd3, xd3, xc3, ALU.add)
            # transpose + argmax
            lab = pool.tile([128, 64], F32, tag="lab", name="lab")
            upf = [u[:].rearrange("p n o -> p (n o)") for u in up]
            for pt in range(64):
                sc = work.tile([128, 256], F32, tag="sc", name="sc")
                for mt in range(2):
                    pst = psum.tile([128, 128], F32, tag="tp", name="tp")
                    nc.tensor.transpose(
                        pst[:], upf[mt][:, pt * 128:(pt + 1) * 128], ident32[:])
                    nc.any.tensor_copy(sc[:, mt * 128:(mt + 1) * 128], pst[:])
                mx = work.tile([128, 8], F32, tag="mx", name="mx")
                nc.vector.max(mx[:], sc[:])
                mi = work.tile([128, 8], U32, tag="mi", name="mi")
                nc.vector.max_index(mi[:], mx[:], sc[:])
                nc.vector.tensor_copy(lab[:, pt:pt + 1], mi[:, 0:1])
            bL_in = dram.tile([128, 64], F32, name="bL_in")
            bL_out = dram.tile([8, 128, 64], F32, name="bL_out")
            nc.sync.dma_start(bL_in[:], lab[:])
            nc.gpsimd.collective_compute(
                "AllGather", ALU.bypass, replica_groups=RG,
                ins=[bL_in.opt()], outs=[bL_out.opt()])
            nc.sync.dma_start(o_lab[:], bL_out[:])

    nc.finalize()
    return nc, ["n2w", "sel3T", "icnt3", "biasv", "idxyc", "idxyd", "wy",
                "idxxc", "idxxd", "wx"], ["lab"]


def _build_fused():
    """Single-launch program: L1 aggregation+merge0, on-device greedy x2,
    L2 merge, L3 upsample+argmax. One host round-trip total."""
    nc, tile, mybir = _build_common()
    from concourse.masks import make_identity
    F32, F16 = mybir.dt.float32, mybir.dt.float16
    U8, I16, U32 = mybir.dt.uint8, mybir.dt.int16, mybir.dt.uint32
    ALU, AX, AF = (mybir.AluOpType, mybir.AxisListType,
                   mybir.ActivationFunctionType)
    RG = [list(range(NCORES))]

    w64 = nc.dram_tensor("w64s", [8, 512, 4096], F32, kind="ExternalInput")
    w32 = nc.dram_tensor("w32s", [8, 256, 1024], F32, kind="ExternalInput")
    w16 = nc.dram_tensor("w16s", [8, 128, 256], F32, kind="ExternalInput")
    w8 = nc.dram_tensor("w8s", [8, 64, 64], F32, kind="ExternalInput")
    iwin = nc.dram_tensor("iwin", [128, 1], I16, kind="ExternalInput")
    idxyc = nc.dram_tensor("idxyc", [128, 2], I16, kind="ExternalInput")
    idxyd = nc.dram_tensor("idxyd", [128, 2], I16, kind="ExternalInput")
    wyr = nc.dram_tensor("wy", [1, 32], F32, kind="ExternalInput")
    idxxc = nc.dram_tensor("idxxc", [128, 512], I16, kind="ExternalInput")
    idxxd = nc.dram_tensor("idxxd", [128, 512], I16, kind="ExternalInput")
    wxr = nc.dram_tensor("wx", [1, 256], F32, kind="ExternalInput")
    o_lab = nc.dram_tensor("lab", [8, 128, 64], U8, kind="ExternalOutput")

    W = L3_W * 64
    with tile.TileContext(nc) as tc:
        with tc.tile_pool(name="consts", bufs=1) as cpool, \
             tc.tile_pool(name="persist", bufs=1) as persist, \
             tc.tile_pool(name="dram", bufs=1, space="DRAM") as dram:

            from concourse.masks import make_identity as _mkid
            ident32 = cpool.tile([128, 128], F32, tag="gid32", name="gid32")
            _mkid(nc, ident32[:])
            ones32 = cpool.tile([128, 1], F32, tag="go32", name="go32")
            nc.gpsimd.memset(ones32[:], 1.0)
            ident16 = cpool.tile([128, 128], F16, tag="id16", name="id16")
            make_identity(nc, ident16[:])
            ones16 = cpool.tile([128, 1], F16, tag="o16", name="o16")
            nc.gpsimd.memset(ones16[:], 1.0)
            bS_out = dram.tile([1, 256], F32, name="bS_out")
            bC1_out = dram.tile([256, 256], F32, name="bC1_out")

            with tc.tile_pool(name="ystore", bufs=1) as ystore:
                Y = [ystore.tile([128, 4096], F32, tag=f"Y{pt}",
                                 name=f"Y{pt}") for pt in range(4)]

                # ============ phases A+B: aggregation ============
                with tc.tile_pool(name="nmap", bufs=1) as nmap, \
                     tc.tile_pool(name="stage", bufs=2) as stage, \
                     tc.tile_pool(name="workab", bufs=1) as workab:

                    def upsample_norm(src_dram, s, P, tidx, out_tile):
                        ssq = s * s
                        m = workab.tile([P, ssq], F32, tag="mA", name="mA")
                        ncb = max(1, ssq // 256)
                        cw = ssq // ncb
                        for cb in range(ncb):
                            stg = stage.tile([P, 8, cw], F32, tag="stg",
                                             name="stg")
                            for h in range(8):
                                nc.sync.dma_start(
                                    stg[:, h, :],
                                    src_dram[h, tidx * P:(tidx + 1) * P,
                                             cb * cw:(cb + 1) * cw])
                            nc.vector.tensor_reduce(
                                m[:, cb * cw:(cb + 1) * cw],
                                stg[:].rearrange("p h x -> p x h"), AX.X,
                                ALU.add)
                        m3 = m[:].rearrange("p (y x) -> p y x", y=s)
                        x0s, wxs = {32: (X032, W32), 16: (X016, W16),
                                    8: (X08, W8)}[s]
                        ux = workab.tile([P, s, 64], F32, tag="uxA",
                                         name="uxA")
                        tmp = workab.tile([P, s], F32, tag="tmpxA",
                                          name="tmpxA")
                        for j in range(64):
                            c = m3[:, :, int(x0s[j])]
                            d = m3[:, :, int(x0s[j]) + 1]
                            eng = nc.vector if j % 2 == 0 else nc.gpsimd
                            eng.tensor_tensor(tmp[:], d, c, ALU.subtract)
                            nc.vector.scalar_tensor_tensor(
                                ux[:, :, j], tmp[:], float(wxs[j]), c,
                                ALU.mult, ALU.add)
                        tmp2 = workab.tile([P, 64], F32, tag="tmpyA",
                                           name="tmpyA")
                        for j in range(64):
                            c = ux[:, int(x0s[j]), :]
                            d = ux[:, int(x0s[j]) + 1, :]
                            eng = nc.vector if j % 2 == 0 else nc.gpsimd
                            eng.tensor_tensor(tmp2[:], d, c, ALU.subtract)
                            nc.vector.scalar_tensor_tensor(
                                out_tile[:, j, :], tmp2[:], float(wxs[j]), c,
                                ALU.mult, ALU.add)
                        rs = workab.tile([P, 1], F32, tag="rsA", name="rsA")
                        nc.vector.tensor_reduce(rs[:], out_tile[:], AX.XY,
                                                ALU.add)
                        nc.vector.reciprocal(rs[:], rs[:])
                        flat = out_tile[:].rearrange("p a b -> p (a b)")
                        nc.vector.tensor_scalar(flat, flat, rs[:], None,
                                                ALU.mult)

                    n32 = [nmap.tile([128, 64, 64], F32, tag=f"n32_{t}",
                                     name=f"n32_{t}") for t in range(2)]
                    for t in range(2):
                        upsample_norm(w32, 32, 128, t, n32[t])
                    n16 = nmap.tile([128, 64, 64], F32, tag="n16", name="n16")
                    upsample_norm(w16, 16, 128, 0, n16)
                    n8 = nmap.tile([64, 64, 64], F32, tag="n8", name="n8")
                    upsample_norm(w8, 8, 64, 0, n8)

                    for pt in range(4):
                        for cb in range(8):
                            stg = stage.tile([128, 8, 512], F32, tag="stg",
                                             name="stg64")
                            for h in range(8):
                                nc.sync.dma_start(
                                    stg[:, h, :],
                                    w64[h, pt * 128:(pt + 1) * 128,
                                        cb * 512:(cb + 1) * 512])
                            nc.vector.tensor_reduce(
                                Y[pt][:, cb * 512:(cb + 1) * 512],
                                stg[:].rearrange("p h x -> p x h"), AX.X,
                                ALU.add)
                        rs = workab.tile([128, 1], F32, tag="rsY", name="rsY")
                        nc.vector.tensor_reduce(rs[:], Y[pt][:], AX.X,
                                                ALU.add)
                        nc.vector.reciprocal(rs[:], rs[:])
                        nc.vector.tensor_scalar(rs[:], rs[:], float(RAT[0]),
                                                None, ALU.mult)
                        nc.vector.tensor_scalar(Y[pt][:], Y[pt][:], rs[:],
                                                None, ALU.mult)
                        rep = workab.tile([128, 4096], F32, tag="rep",
                                          name="rep")
                        srct = n32[pt // 2]
                        base = (2 * pt) % 4 * 32
                        for ar in range(2):
                            for rp in range(2):
                                nc.sync.dma_start(
                                    rep[ar * 64 + rp * 32:
                                        ar * 64 + rp * 32 + 32, :],
                                    srct[base + ar * 32:base + ar * 32 + 32,
                                         :, :]
                                    .rearrange("p a b -> p (a b)"))
                        nc.vector.scalar_tensor_tensor(
                            Y[pt][:], rep[:], float(RAT[1]), Y[pt][:],
                            ALU.mult, ALU.add)
                        rep2 = workab.tile([128, 4096], F32, tag="rep",
                                           name="rep2")
                        for ar in range(2):
                            a_loc = 2 * pt + ar
                            for rp in range(4):
                                nc.sync.dma_start(
                                    rep2[ar * 64 + rp * 16:
                                         ar * 64 + rp * 16 + 16, :],
                                    n16[a_loc * 16:a_loc * 16 + 16, :, :]
                                    .rearrange("p a b -> p (a b)"))
                        nc.vector.scalar_tensor_tensor(
                            Y[pt][:], rep2[:], float(RAT[2]), Y[pt][:],
                            ALU.mult, ALU.add)
                        rep3 = workab.tile([128, 4096], F32, tag="rep",
                                           name="rep3")
                        for ar in range(2):
                            a_loc = 2 * pt + ar
                            for rp in range(8):
                                nc.sync.dma_start(
                                    rep3[ar * 64 + rp * 8:
                                         ar * 64 + rp * 8 + 8, :],
                                    n8[(a_loc % 8) * 8:(a_loc % 8) * 8 + 8,
                                       :, :]
                                    .rearrange("p a b -> p (a b)"))
                        nc.vector.scalar_tensor_tensor(
                            Y[pt][:], rep3[:], float(RAT[3]), Y[pt][:],
                            ALU.mult, ALU.add)

                # ============ phase C: merge0 ============
                with tc.tile_pool(name="xstore", bufs=1) as xstore, \
                     tc.tile_pool(name="workc", bufs=2) as workc:
                    yh_nat = [xstore.tile([128, 4096], F16, tag=f"yhn{pt}",
                                          name=f"yhn{pt}") for pt in range(4)]
                    sycol = [cpool.tile([128, 1], F32, tag=f"syc{mt}",
                                        name=f"syc{mt}") for mt in range(4)]
                    for pt in range(4):
                        nc.scalar.activation(yh_nat[pt][:], Y[pt][:], AF.Copy)
                        lnn = workc.tile([128, 4096], F16, tag="lnn",
                                         name="lnn", bufs=1)
                        nc.scalar.activation(lnn[:], yh_nat[pt][:], AF.Ln)
                        nc.vector.tensor_tensor(lnn[:], yh_nat[pt][:], lnn[:],
                                                ALU.mult)
                        nc.vector.tensor_reduce(sycol[pt][:], lnn[:], AX.X,
                                                ALU.add)

                    psA_cm = tc.tile_pool(name="psA", bufs=2, space="PSUM")
                    psA = psA_cm.__enter__()
                    psAs_cm = tc.tile_pool(name="psAs", bufs=1, space="PSUM")
                    psAs = psAs_cm.__enter__()
                    xloc = workc.tile([32, 4096], F32, tag="xloc",
                                      name="xloc", bufs=1)
                    nc.sync.dma_start(xloc[0:16, :], Y[0][65:126:4, :])
                    nc.sync.dma_start(xloc[16:32, :], Y[2][65:126:4, :])
                    xloch = workc.tile([32, 4096], F16, tag="xloch",
                                       name="xloch", bufs=1)
                    nc.scalar.activation(xloch[:], xloc[:], AF.Copy)
                    bx_in = dram.tile([32, 4096], F16, name="bx_in")
                    bx_out = dram.tile([8, 32, 4096], F16, name="bx_out")
                    nc.sync.dma_start(bx_in[:], xloch[:])
                    nc.gpsimd.collective_compute(
                        "AllGather", ALU.bypass, replica_groups=RG,
                        ins=[bx_in.opt()], outs=[bx_out.opt()])
                    xT = xstore.tile([128, 32, 256], F16, tag="xT", name="xT")
                    lxT = xstore.tile([128, 32, 256], F16, tag="lxT",
                                      name="lxT")
                    bxv = bx_out[:].rearrange("c a p -> (c a) p")
                    for nt in range(2):
                        xnat = workc.tile([128, 4096], F16, tag="xnat",
                                          name="xnat", bufs=1)
                        nc.sync.dma_start(xnat[:],
                                          bxv[nt * 128:(nt + 1) * 128, :])
                        for ct in range(32):
                            pst = psA.tile([128, 128], F16, tag="tp16",
                                           name="tp16")
                            nc.tensor.transpose(
                                pst[:], xnat[:, ct * 128:(ct + 1) * 128],
                                ident16[:])
                            nc.any.tensor_copy(
                                xT[:, ct, nt * 128:(nt + 1) * 128], pst[:])
                    for ct in range(32):
                        nc.scalar.activation(lxT[:, ct, :], xT[:, ct, :],
                                             AF.Ln)
                    sxP = psAs.tile([1, 256], F32, tag="sx", name="sxP")
                    for ct in range(32):
                        prodx = workc.tile([128, 256], F16, tag="prodX",
                                           name="prodX")
                        nc.vector.tensor_tensor(prodx[:], xT[:, ct, :],
                                                lxT[:, ct, :], ALU.mult)
                        nc.tensor.matmul(sxP[:], ones16[:], prodx[:],
                                         start=(ct == 0), stop=(ct == 31))
                    sx_sb = workc.tile([1, 256], F32, tag="sxsb", name="sxsb",
                                       bufs=1)
                    nc.any.tensor_copy(sx_sb[:], sxP[:])
                    sxb = xstore.tile([128, 256], F32, tag="sxb", name="sxb")
                    nc.gpsimd.partition_broadcast(sxb[:], sx_sb[:])

                    psC = [psA.tile([128, 256], F32, tag=f"psC{mt}",
                                    name=f"psC{mt}", bufs=1)
                           for mt in range(4)]
                    for ct in range(32):
                        yhTct = workc.tile([128, 512], F16, tag="yhTct",
                                           name="yhTct")
                        for pt in range(4):
                            pst = psA.tile([128, 128], F16, tag="tp16",
                                           name="tp16b")
                            nc.tensor.transpose(
                                pst[:],
                                yh_nat[pt][:, ct * 128:(ct + 1) * 128],
                                ident16[:])
                            nc.any.tensor_copy(
                                yhTct[:, pt * 128:(pt + 1) * 128], pst[:])
                        lnct = workc.tile([128, 512], F16, tag="lnct",
                                          name="lnct")
                        nc.scalar.activation(lnct[:], yhTct[:], AF.Ln)
                        for mt in range(4):
                            nc.tensor.matmul(
                                psC[mt][:], lnct[:, mt * 128:(mt + 1) * 128],
                                xT[:, ct, :], start=(ct == 0), stop=False)
                            nc.tensor.matmul(
                                psC[mt][:], yhTct[:, mt * 128:(mt + 1) * 128],
                                lxT[:, ct, :], start=False, stop=(ct == 31))
                    knT = xstore.tile([128, 4, 256], F32, tag="knT",
                                      name="knT")
                    for mt in range(4):
                        S = workc.tile([128, 256], F32, tag="Ssum",
                                       name="Ssum")
                        nc.vector.tensor_scalar(S[:], sxb[:], sycol[mt][:],
                                                None, ALU.add)
                        nc.vector.tensor_tensor(S[:], S[:], psC[mt][:],
                                                ALU.subtract)
                        nc.vector.tensor_scalar(knT[:, mt, :], S[:],
                                                float(THR2), None, ALU.is_lt)

                    psAs_cm.__exit__(None, None, None)
                    psA_cm.__exit__(None, None, None)
                    psB_cm = tc.tile_pool(name="psB", bufs=2, space="PSUM")
                    psB = psB_cm.__enter__()
                    psBs_cm = tc.tile_pool(name="psBs", bufs=1, space="PSUM")
                    psBs = psBs_cm.__enter__()
                    cntP = psBs.tile([1, 256], F32, tag="cnt", name="cntP")
                    for mt in range(4):
                        nc.tensor.matmul(cntP[:], ones32[:], knT[:, mt, :],
                                         start=(mt == 0), stop=(mt == 3))
                    bN_in = dram.tile([4096, 256], F32, name="bN_in")
                    for ptile in range(32):
                        psN = psB.tile([128, 256], F32, tag="psN", name="psN")
                        for mt in range(4):
                            nc.tensor.matmul(
                                psN[:],
                                Y[mt][:, ptile * 128:(ptile + 1) * 128],
                                knT[:, mt, :], start=(mt == 0), stop=(mt == 3))
                        npt = workc.tile([128, 256], F32, tag="npt",
                                         name="npt")
                        nc.any.tensor_copy(npt[:], psN[:])
                        nc.sync.dma_start(
                            bN_in[ptile * 128:(ptile + 1) * 128, :], npt[:])
                    bC_in = dram.tile([1, 256], F32, name="bC_in")
                    bC_out = dram.tile([1, 256], F32, name="bC_out")
                    cnt_sb = workc.tile([1, 256], F32, tag="cntsb",
                                        name="cntsb", bufs=1)
                    nc.any.tensor_copy(cnt_sb[:], cntP[:])
                    nc.sync.dma_start(bC_in[:], cnt_sb[:])
                    nc.gpsimd.collective_compute(
                        "AllReduce", ALU.add, replica_groups=RG,
                        ins=[bC_in.opt()], outs=[bC_out.opt()])
                    bN_out = dram.tile([512, 256], F32, name="bN_out")
                    nc.gpsimd.collective_compute(
                        "ReduceScatter", ALU.add, replica_groups=RG,
                        ins=[bN_in.opt()], outs=[bN_out.opt()])

                    cntg = workc.tile([1, 256], F32, tag="cntg", name="cntg",
                                      bufs=1)
                    nc.sync.dma_start(cntg[:], bC_out[:])
                    nc.vector.reciprocal(cntg[:], cntg[:])
                    cb2 = xstore.tile([128, 256], F32, tag="cb2", name="cb2")
                    nc.gpsimd.partition_broadcast(cb2[:], cntg[:])
                    x2T = [xstore.tile([128, 256], F32, tag=f"x2T{t}",
                                       name=f"x2T{t}") for t in range(4)]
                    lx2T = [xstore.tile([128, 256], F32, tag=f"lx2T{t}",
                                        name=f"lx2T{t}") for t in range(4)]
                    for t in range(4):
                        nc.sync.dma_start(x2T[t][:],
                                          bN_out[t * 128:(t + 1) * 128, :])
                        nc.vector.tensor_tensor(x2T[t][:], x2T[t][:], cb2[:],
                                                ALU.mult)
                        nc.scalar.activation(lx2T[t][:], x2T[t][:], AF.Ln)
                    sx2P = psBs.tile([1, 256], F32, tag="sx2", name="sx2P")
                    for t in range(4):
                        prod2 = workc.tile([128, 256], F32, tag="prod2",
                                           name="prod2")
                        nc.vector.tensor_tensor(prod2[:], x2T[t][:],
                                                lx2T[t][:], ALU.mult)
                        nc.tensor.matmul(sx2P[:], ones32[:], prod2[:],
                                         start=(t == 0), stop=(t == 3))
                    sx2sb = workc.tile([1, 256], F32, tag="sx2sb",
                                       name="sx2sb", bufs=1)
                    nc.any.tensor_copy(sx2sb[:], sx2P[:])
                    bS_in = dram.tile([1, 256], F32, name="bS_in")
                    nc.sync.dma_start(bS_in[:], sx2sb[:])
                    nc.gpsimd.collective_compute(
                        "AllReduce", ALU.add, replica_groups=RG,
                        ins=[bS_in.opt()], outs=[bS_out.opt()])
                    bC1_in = dram.tile([256, 256], F32, name="bC1_in")
                    for it in range(2):
                        psC1 = psB.tile([128, 256], F32, tag="pc1",
                                        name="pc1")
                        for kt in range(4):
                            nc.tensor.matmul(
                                psC1[:], x2T[kt][:, it * 128:(it + 1) * 128],
                                lx2T[kt][:], start=(kt == 0), stop=(kt == 3))
                        c1t = workc.tile([128, 256], F32, tag="c1t",
                                         name="c1t")
                        nc.any.tensor_copy(c1t[:], psC1[:])
                        nc.sync.dma_start(bC1_in[it * 128:(it + 1) * 128, :],
                                          c1t[:])
                    nc.gpsimd.collective_compute(
                        "AllReduce", ALU.add, replica_groups=RG,
                        ins=[bC1_in.opt()], outs=[bC1_out.opt()])
                    xs = [persist.tile([128, 512], F32, tag=f"xs{t}",
                                       name=f"xs{t}") for t in range(2)]
                    for it in range(2):
                        for kt in range(4):
                            pst = psB.tile([128, 128], F32, tag="tp32",
                                           name="tp32")
                            nc.tensor.transpose(
                                pst[:], x2T[kt][:, it * 128:(it + 1) * 128],
                                ident32[:])
                            nc.any.tensor_copy(
                                xs[it][:, kt * 128:(kt + 1) * 128], pst[:])
                    psBs_cm.__exit__(None, None, None)
                    psB_cm.__exit__(None, None, None)

            # ============ greedy 1 + L2 + greedy 2 ============
            with tc.tile_pool(name="gpool", bufs=1) as gpool, \
                 tc.tile_pool(name="gwork", bufs=2) as gwork:
                G = _emit_greedy_consts(nc, tile, mybir, gpool,
                                        ident32=ident32, ones32=ones32)
                zrow, iocol = G["zrow"], G["iocol"]
                # L3 constants
                iw = gpool.tile([128, 1], I16, tag="iw", name="iw")
                nc.sync.dma_start(iw[:], iwin[:])
                iyc = gpool.tile([128, 2], I16, tag="iyc", name="iyc")
                iyd = gpool.tile([128, 2], I16, tag="iyd", name="iyd")
                ixc = gpool.tile([128, 512], I16, tag="ixc", name="ixc")
                ixd = gpool.tile([128, 512], I16, tag="ixd", name="ixd")
                for t_, s_ in ((iyc, idxyc), (iyd, idxyd), (ixc, idxxc),
                               (ixd, idxxd)):
                    nc.sync.dma_start(t_[:], s_[:])
                wyt = gpool.tile([128, 32], F32, tag="wyt", name="wyt")
                wxt = gpool.tile([128, 256], F32, tag="wxt", name="wxt")
                wsb = gpool.tile([1, 32], F32, tag="wsb", name="wsb")
                nc.sync.dma_start(wsb[:], wyr[:])
                nc.gpsimd.partition_broadcast(wyt[:], wsb[:])
                wsb2 = gpool.tile([1, 256], F32, tag="wsb2", name="wsb2")
                nc.sync.dma_start(wsb2[:], wxr[:])
                nc.gpsimd.partition_broadcast(wxt[:], wsb2[:])
                nw = [gpool.tile([128, W], F32, tag=f"nw{t}",
                                 name=f"nw{t}") for t in range(2)]
                psG_cm = tc.tile_pool(name="psG", bufs=2, space="PSUM")
                psG = psG_cm.__enter__()
                psGS_cm = tc.tile_pool(name="psGS", bufs=1, space="PSUM")
                psGS = psGS_cm.__enter__()
                with tc.tile_pool(name="gscr", bufs=1) as gscr, \
                     tc.tile_pool(name="gw1", bufs=1) as gw1:
                    G1 = _emit_greedy(nc, tc, mybir, G, gpool, gw1, psG,
                                      psGS, gscr, dram, bC1_out[:, :],
                                      bS_out[:, :], zrow[:], "g1")
                # new2 = sel2 @ X2 / cnt   (local 512-pixel slice)
                new2 = [gpool.tile([128, 512], F32, tag=f"n2{t}",
                                   name=f"n2{t}") for t in range(2)]
                for mt in range(2):
                    ps = psG.tile([128, 512], F32, tag="psn2", name="psn2")
                    for kt in range(2):
                        nc.tensor.matmul(
                            ps[:], G1["selT"][kt][:, mt * 128:(mt + 1) * 128],
                            xs[kt][:], start=(kt == 0), stop=(kt == 1))
                    nc.vector.tensor_scalar(new2[mt][:], ps[:],
                                            G1["rc"][mt][:], None, ALU.mult)
                bG_in = dram.tile([256, 512], F32, name="bG_in")
                bG_out = dram.tile([8, 256, 512], F32, name="bG_out")
                for t in range(2):
                    nc.sync.dma_start(bG_in[t * 128:(t + 1) * 128, :],
                                      new2[t][:])
                nc.gpsimd.collective_compute(
                    "AllGather", ALU.bypass, replica_groups=RG,
                    ins=[bG_in.opt()], outs=[bG_out.opt()])
                with tc.tile_pool(name="n2fp", bufs=1) as n2fp:
                    for rt in range(2):
                        n2full = n2fp.tile([128, 64, 64], F32, tag="n2full",
                                           name="n2full")
                        n2fv = n2full[:].rearrange("p a b -> p (a b)")
                        for k in range(NCORES):
                            nc.sync.dma_start(
                                n2fv[:, k * 512:(k + 1) * 512],
                                bG_out[k, rt * 128:(rt + 1) * 128, :])
                        nwv = nw[rt][:].rearrange("p (a b) -> p a b", a=L3_W)
                        nc.gpsimd.ap_gather(nwv, n2full[:], iw[:],
                                            channels=128, num_elems=64,
                                            d=64, num_idxs=L3_W)
                # n2T + masked ln + sx3/C3 partials
                n2T = [gpool.tile([128, 256], F32, tag=f"n2T{t}",
                                  name=f"n2T{t}") for t in range(4)]
                for ct in range(4):
                    for rt in range(2):
                        pst = psG.tile([128, 128], F32, tag="gtp", name="tpg")
                        nc.tensor.transpose(
                            pst[:], new2[rt][:, ct * 128:(ct + 1) * 128],
                            ident32[:])
                        nc.any.tensor_copy(
                            n2T[ct][:, rt * 128:(rt + 1) * 128], pst[:])
                ib_row = gpool.tile([1, 256], F32, tag="ibr", name="ibr")
                nc.vector.tensor_scalar(ib_row[:], G1["valid_row"][:], 0.5,
                                        None, ALU.is_lt)
                vb = gpool.tile([128, 256], F32, tag="vb", name="vb")
                ibb = gpool.tile([128, 256], F32, tag="ibb", name="ibb")
                nc.gpsimd.partition_broadcast(vb[:], G1["valid_row"][:])
                nc.gpsimd.partition_broadcast(ibb[:], ib_row[:])
                ln2T = [gpool.tile([128, 256], F32, tag=f"ln2T{t}",
                                   name=f"ln2T{t}") for t in range(4)]
                sx3P = psGS.tile([1, 256], F32, tag="sx3", name="sx3")
                for ct in range(4):
                    msk = gwork.tile([128, 256], F32, tag="msk", name="msk")
                    nc.vector.tensor_tensor(msk[:], n2T[ct][:], vb[:],
                                            ALU.mult)
                    nc.vector.tensor_tensor(msk[:], msk[:], ibb[:], ALU.add)
                    nc.scalar.activation(ln2T[ct][:], msk[:], AF.Ln)
                    prod = gwork.tile([128, 256], F32, tag="prod",
                                      name="prod")
                    nc.vector.tensor_tensor(prod[:], n2T[ct][:], ln2T[ct][:],
                                            ALU.mult)
                    nc.tensor.matmul(sx3P[:], ones32[:], prod[:],
                                     start=(ct == 0), stop=(ct == 3))
                sx3sb = gwork.tile([1, 256], F32, tag="sx3sb", name="sx3sb",
                                   bufs=1)
                nc.any.tensor_copy(sx3sb[:], sx3P[:])
                bS3_in = dram.tile([1, 256], F32, name="bS3_in")
                bS3_out = dram.tile([1, 256], F32, name="bS3_out")
                nc.sync.dma_start(bS3_in[:], sx3sb[:])
                nc.gpsimd.collective_compute(
                    "AllReduce", ALU.add, replica_groups=RG,
                    ins=[bS3_in.opt()], outs=[bS3_out.opt()])
                bC3_in = dram.tile([256, 256], F32, name="bC3_in")
                bC3_out = dram.tile([256, 256], F32, name="bC3_out")
                for it in range(2):
                    psC3 = psG.tile([128, 256], F32, tag="gps", name="psC3")
                    for kt in range(4):
                        nc.tensor.matmul(
                            psC3[:], n2T[kt][:, it * 128:(it + 1) * 128],
                            ln2T[kt][:], start=(kt == 0), stop=(kt == 3))
                    c3t = gwork.tile([128, 256], F32, tag="c3t", name="c3t")
                    nc.any.tensor_copy(c3t[:], psC3[:])
                    nc.sync.dma_start(bC3_in[it * 128:(it + 1) * 128, :],
                                      c3t[:])
                nc.gpsimd.collective_compute(
                    "AllReduce", ALU.add, replica_groups=RG,
                    ins=[bC3_in.opt()], outs=[bC3_out.opt()])
                with tc.tile_pool(name="gscr2", bufs=1) as gscr2, \
                     tc.tile_pool(name="gw2", bufs=1) as gw2:
                    G2 = _emit_greedy(nc, tc, mybir, G, gpool, gw2, psG,
                                      psGS, gscr2, dram, bC3_out[:, :],
                                      bS3_out[:, :], ib_row[:], "g2")
                # bias columns for invalid output rows
                oc3b = gpool.tile([128, 1], F32, tag="oc3b", name="oc3b")
                nc.gpsimd.partition_broadcast(oc3b[:], G2["oc"][:])
                bias = [gpool.tile([128, 1], F32, tag=f"bias{t}",
                                   name=f"bias{t}") for t in range(2)]
                for t in range(2):
                    nc.vector.tensor_scalar(bias[t][:], iocol[t][:],
                                            oc3b[:], None, ALU.is_ge)
                    nc.vector.tensor_scalar(bias[t][:], bias[t][:],
                                            float(NEG_BIG), None, ALU.mult)
                psGS_cm.__exit__(None, None, None)
                psG_cm.__exit__(None, None, None)

                # ============ L3: merge3 + upsample + argmax ============
                with tc.tile_pool(name="big", bufs=1) as big, \
                     tc.tile_pool(name="lw1", bufs=1) as lw1, \
                     tc.tile_pool(name="lwork", bufs=2) as lwork, \
                     tc.tile_pool(name="lps", bufs=2, space="PSUM") as lps:
                    up = [big.tile([128, 8192, 1], F32, tag=f"up{t}",
                                   name=f"up{t}") for t in range(2)]
                    for mt in range(2):
                        n3 = lw1.tile([128, W], F32, tag="n3", name="n3")
                        for half, (c0, c1) in enumerate(((0, 512), (512, W))):
                            ps = lps.tile([128, c1 - c0], F32,
                                          tag=f"ps{half}", name=f"ps{half}")
                            for kt in range(2):
                                nc.tensor.matmul(
                                    ps[:],
                                    G2["selT"][kt][:, mt * 128:(mt + 1) * 128],
                                    nw[kt][:, c0:c1],
                                    start=(kt == 0), stop=(kt == 1))
                            nc.vector.tensor_scalar(n3[:, c0:c1], ps[:],
                                                    G2["rc"][mt][:], None,
                                                    ALU.mult)
                        nc.vector.tensor_scalar(n3[:], n3[:], bias[mt][:],
                                                None, ALU.add)
                        yc = lw1.tile([128, 32, 64], F32, tag="yc", name="yc")
                        yd = lw1.tile([128, 32, 64], F32, tag="yd", name="yd")
                        ydr = lw1.tile([128, 2048, 1], F32, tag="ydr",
                                       name="ydr")
                        n3v = n3[:].rearrange("p (y x) -> p y x", y=L3_W)
                        nc.gpsimd.ap_gather(yc[:], n3v, iyc[:], channels=128,
                                            num_elems=L3_W, d=64, num_idxs=32)
                        nc.gpsimd.ap_gather(yd[:], n3v, iyd[:], channels=128,
                                            num_elems=L3_W, d=64, num_idxs=32)
                        yc3 = yc[:]
                        yd3 = yd[:]
                        ydr3 = ydr[:].rearrange("p (y x) o -> p y (x o)",
                                                y=32)
                        wy3 = wyt[:, :, None].broadcast_to([128, 32, 64])
                        nc.vector.tensor_tensor(ydr3, yd3, yc3, ALU.subtract)
                        nc.vector.tensor_tensor(ydr3, ydr3, wy3, ALU.mult)
                        nc.vector.tensor_tensor(ydr3, ydr3, yc3, ALU.add)
                        xc = big.tile([128, 8192, 1], F32, tag="xc",
                                      name="xc")
                        xd = up[mt]
                        nc.gpsimd.ap_gather(xc[:], ydr[:], ixc[:],
                                            channels=128, num_elems=2048,
                                            d=1, num_idxs=8192)
                        nc.gpsimd.ap_gather(xd[:], ydr[:], ixd[:],
                                            channels=128, num_elems=2048,
                                            d=1, num_idxs=8192)
                        xc3 = xc[:].rearrange("p (y j) o -> p y (j o)", y=32)
                        xd3 = xd[:].rearrange("p (y j) o -> p y (j o)", y=32)
                        wx3 = wxt[:, None, :].broadcast_to([128, 32, 256])
                        nc.vector.tensor_tensor(xd3, xd3, xc3, ALU.subtract)
                        nc.vector.tensor_tensor(xd3, xd3, wx3, ALU.mult)
                        nc.vector.tensor_tensor(xd3, xd3, xc3, ALU.add)
                    lab = lw1.tile([128, 64], F32, tag="lab", name="lab")
                    upf = [u[:].rearrange("p n o -> p (n o)") for u in up]
                    for pt in range(64):
                        sc = lwork.tile([128, 256], F32, tag="sc", name="sc")
                        for mt in range(2):
                            pst = lps.tile([128, 128], F32, tag="tp",
                                           name="tp")
                            nc.tensor.transpose(
                                pst[:], upf[mt][:, pt * 128:(pt + 1) * 128],
                                ident32[:])
                            nc.any.tensor_copy(
                                sc[:, mt * 128:(mt + 1) * 128], pst[:])
                        mx = lwork.tile([128, 8], F32, tag="mx", name="mx")
                        nc.vector.max(mx[:], sc[:])
                        mi = lwork.tile([128, 8], U32, tag="mi", name="mi")
                        nc.vector.max_index(mi[:], mx[:], sc[:])
                        nc.vector.tensor_copy(lab[:, pt:pt + 1], mi[:, 0:1])
                    labu = lw1.tile([128, 64], U8, tag="labu", name="labu")
                    nc.vector.tensor_copy(labu[:], lab[:])
                    bL_in = dram.tile([128, 64], U8, name="bL_in")
                    bL_out = dram.tile([8, 128, 64], U8, name="bL_out")
                    nc.sync.dma_start(bL_in[:], labu[:])
                    nc.gpsimd.collective_compute(
                        "AllGather", ALU.bypass, replica_groups=RG,
                        ins=[bL_in.opt()], outs=[bL_out.opt()])
                    nc.sync.dma_start(o_lab[:], bL_out[:])

    nc.finalize()
    return nc, ["w64s", "w32s", "w16s", "w8s", "iwin", "idxyc", "idxyd",
                "wy", "idxxc", "idxxd", "wx"], ["lab"]


# ------------------------------------------------------------------- runner
class _Runner:
    """Cached shard_map-jitted executor for a finalized Bass program
    (modeled on bass2jax.run_bass_via_pjrt, but reusable across calls)."""

    def __init__(self, nc):
        import jax
        import jax.numpy as jnp
        from jax.sharding import Mesh, PartitionSpec, NamedSharding
        from jax.experimental.shard_map import shard_map
        from concourse import bass2jax as b2j
        from concourse import mybir
        b2j.install_neuronx_cc_hook()
        self.jax = jax
        self.np_outs = []
        in_names, out_names, out_avals, zero_outs = [], [], [], []
        partition_name = (nc.partition_id_tensor.name
                          if nc.partition_id_tensor else None)
        for alloc in nc.m.functions[0].allocations:
            if not isinstance(alloc, mybir.MemoryLocationSet):
                continue
            name = alloc.memorylocations[0].name
            if alloc.kind == "ExternalInput":
                if name != partition_name:
                    in_names.append(name)
            elif alloc.kind == "ExternalOutput":
                shape = tuple(alloc.tensor_shape)
                dtype = mybir.dt.np(alloc.dtype)
                out_names.append(name)
                out_avals.append(jax.core.ShapedArray(shape, dtype))
                zero_outs.append(np.zeros(shape, dtype))
        self.in_names, self.out_names = in_names, out_names
        self.zero_outs = zero_outs
        n_params = len(in_names)
        bind_in_names = tuple(in_names + out_names +
                              ([partition_name] if partition_name else []))

        def _body(*args):
            operands = list(args)
            if partition_name is not None:
                operands.append(b2j.partition_id_tensor())
            outs = b2j._bass_exec_p.bind(
                *operands,
                out_avals=tuple(out_avals),
                in_names=bind_in_names,
                out_names=tuple(out_names),
                lowering_input_output_aliases=(),
                sim_require_finite=False,
                sim_require_nnan=False,
                nc=nc,
            )
            return tuple(outs)

        devices = jax.devices()[:NCORES]
        mesh = Mesh(np.asarray(devices), ("core",))
        n_outs = len(out_names)
        in_specs = (PartitionSpec("core"),) * (n_params + n_outs)
        out_specs = (PartitionSpec("core"),) * n_outs
        self.fn = jax.jit(
            shard_map(_body, mesh=mesh, in_specs=in_specs,
                      out_specs=out_specs, check_rep=False),
            donate_argnums=tuple(range(n_params, n_params + n_outs)),
            keep_unused=True)
        self.out_avals = out_avals
        # donated zero output buffers, created on-device (no H2D)
        zsh = NamedSharding(mesh, PartitionSpec("core"))
        zspecs = [((NCORES * z.shape[0], *z.shape[1:]), z.dtype)
                  for z in zero_outs]
        self.zfn = jax.jit(
            lambda: tuple(jnp.zeros(s, d) for s, d in zspecs),
            out_shardings=tuple(zsh for _ in zspecs))
        self.in_sharding = zsh

    def __call__(self, per_core_maps):
        concat_in = [np.concatenate([np.asarray(per_core_maps[c][nm])
                                     for c in range(NCORES)], axis=0)
                     for nm in self.in_names]
        return self.run_concat(concat_in)

    def run_raw(self, concat_in, zeros=None):
        """concat_in: list of [NCORES*s0, ...] arrays (np or device jax).
        Returns tuple of sharded jax output arrays. Pass pre-issued `zeros`
        (from self.zfn()) to overlap zero-buffer creation with earlier work."""
        return self.fn(*concat_in, *(zeros if zeros is not None
                                     else self.zfn()))

    def run_concat(self, concat_in):
        out = self.run_raw(concat_in)
        res = []
        for c in range(NCORES):
            res.append({nm: np.asarray(out[i]).reshape(
                NCORES, *self.out_avals[i].shape)[c]
                for i, nm in enumerate(self.out_names)})
        return res


def _get_runner(name):
    if name not in _PROGS:
        build = {"l1": _build_l1, "l2": _build_l2, "l3": _build_l3,
                 "fused": _build_fused}[name]
        nc, ins, outs = build()
        _PROGS[name] = _Runner(nc)
    return _PROGS[name]


_FUSED_CONSTS = {}


def _fused_const_dev():
    """Per-core constant inputs for the fused program, staged on device once."""
    if not _FUSED_CONSTS:
        import jax
        from jax.sharding import Mesh, PartitionSpec, NamedSharding
        mesh = Mesh(np.asarray(jax.devices()[:NCORES]), ("core",))
        sh = NamedSharding(mesh, PartitionSpec("core"))
        _FUSED_CONSTS.update(_l3_const_dev())
        _FUSED_CONSTS["iwin"] = jax.device_put(_IWIN_CAT, sh)
    return _FUSED_CONSTS


def _segment_one_fused(w64, w32, w16, w8, l1_dev_in=None):
    r = _get_runner("fused")
    per = dict(_fused_const_dev())
    weights = (l1_dev_in if l1_dev_in is not None
               else _prep_l1_inputs(w64, w32, w16, w8))
    for nm, v in zip(("w64s", "w32s", "w16s", "w8s"), weights):
        per[nm] = v
    raw = r.run_raw([per[nm] for nm in r.in_names])
    lab = np.asarray(
        raw[r.out_names.index("lab")].addressable_shards[0].data)
    out = np.empty((65536,), np.int32)
    for k in range(NCORES):
        out[8192 * k:8192 * (k + 1)] = lab[k].T.reshape(-1).astype(np.int32)
    return out.reshape(256, 256)


# ------------------------------------------------------------------- host math
def _greedy(klmat, valid):
    """Reference greedy loop via 256-bit ints. Returns sel bool [256,256], oc."""
    N = klmat.shape[0]
    adj = (klmat < np.float32(0.9)) & valid[None, :]
    rows = [int.from_bytes(np.packbits(adj[i], bitorder='little').tobytes(),
                           'little') for i in range(N)]
    vbits = int.from_bytes(np.packbits(valid, bitorder='little').tobytes(),
                           'little')
    matched = 0
    sel_rows = []
    for i in range(N):
        if (vbits >> i) & 1 and not (matched >> i) & 1:
            matched |= rows[i]
            sel_rows.append(rows[i])
    sel = np.zeros((N, N), bool)
    for o, r in enumerate(sel_rows):
        sel[o] = np.unpackbits(
            np.frombuffer(r.to_bytes(32, 'little'), np.uint8),
            bitorder='little')[:N]
    return sel, len(sel_rows)


def _klmat_host(sx, C):
    """0.5*(((sx_i+sx_j) - C) - C.T) in f32, matching the reference order."""
    t = (sx[:, None] + sx[None, :]).astype(np.float32)
    t = t - C
    t = t - C.T
    return (np.float32(0.5) * t).astype(np.float32)


def _prep_l1_inputs(w64, w32, w16, w8):
    cat64 = np.empty((64, 512, 4096), np.float32)
    cat32 = np.empty((64, 256, 1024), np.float32)
    cat16 = np.empty((64, 128, 256), np.float32)
    cat8 = np.empty((64, 64, 64), np.float32)
    for k in range(NCORES):
        cat64[8 * k:8 * k + 8] = w64[:, 512 * k:512 * k + 512, :]
        r32 = (8 * k) % 32 * 32
        cat32[8 * k:8 * k + 8] = w32[:, r32:r32 + 256, :]
        r16 = (8 * k) % 16 * 16
        cat16[8 * k:8 * k + 8] = w16[:, r16:r16 + 128, :]
        cat8[8 * k:8 * k + 8] = w8
    return [cat64, cat32, cat16, cat8]


def _segment_one(w64, w32, w16, w8, l1_dev_in=None):
    r1 = _get_runner("l1")
    r2 = _get_runner("l2")
    r3 = _get_runner("l3")
    raw1 = r1.run_raw(l1_dev_in if l1_dev_in is not None
                      else _prep_l1_inputs(w64, w32, w16, w8))
    # issue L2/L3 donated-zero creation now: overlaps L1 execution
    z2 = r2.zfn()
    z3 = r3.zfn()
    x2_dev = raw1[r1.out_names.index("x2slice")]   # [2048,512] sharded
    # klpack is AllReduced on device -> fetch core 0's shard only (1 RTT)
    klp = np.asarray(raw1[r1.out_names.index("klpack")]
                     .addressable_shards[0].data)
    C1, sx2 = klp[0:256], klp[256]
    valid = np.ones(256, bool)
    klmat2 = _klmat_host(sx2, C1)
    klmat2 = np.where(valid[None, :], klmat2, np.float32(np.inf))
    sel2, oc2 = _greedy(klmat2, valid)
    sel2f = sel2.astype(np.float32)
    cnt2 = np.maximum(sel2f.sum(1), 1.0).astype(np.float32)
    valid2 = (np.arange(256) < oc2)

    sel2T = np.ascontiguousarray(sel2f.T.astype(np.uint8))
    vrow = valid2.astype(np.float32)[None, :]
    irow = (1.0 - vrow).astype(np.float32)
    per_name2 = {"x2s": x2_dev,                       # device-chained from L1
                 "sel2T": np.tile(sel2T, (NCORES, 1)),
                 "icnt2": np.tile(cnt2[:, None], (NCORES, 1)),
                 "vrow": np.tile(vrow, (NCORES, 1)),
                 "irow": np.tile(irow, (NCORES, 1)),
                 "iwin": _IWIN_CAT}
    raw2 = r2.run_raw([per_name2[nm] for nm in r2.in_names], zeros=z2)
    n2w_dev = raw2[r2.out_names.index("n2w")]      # [2048,768] sharded
    klp3 = np.asarray(raw2[r2.out_names.index("klpack3")]
                      .addressable_shards[0].data)
    C3, sx3 = klp3[0:256], klp3[256]
    klmat3 = _klmat_host(sx3, C3)
    klmat3 = np.where(valid2[None, :], klmat3, np.float32(np.inf))
    sel3, oc3 = _greedy(klmat3, valid2)
    sel3f = sel3.astype(np.float32)
    cnt3 = np.maximum(sel3f.sum(1), 1.0).astype(np.float32)
    valid3 = (np.arange(256) < oc3)

    sel3T = np.ascontiguousarray(sel3f.T.astype(np.uint8))
    biasv = np.where(valid3, np.float32(0.0), NEG_BIG).astype(np.float32)[:, None]
    per_name3 = {"n2w": n2w_dev,                      # device-chained from L2
                 "sel3T": np.tile(sel3T, (NCORES, 1)),
                 "icnt3": np.tile(cnt3[:, None], (NCORES, 1)),
                 "biasv": np.tile(biasv, (NCORES, 1))}
    per_name3.update(_l3_const_dev())
    raw3 = r3.run_raw([per_name3[nm] for nm in r3.in_names], zeros=z3)
    # lab is AllGathered on device: fetch core 0's shard [8, 128, 64] only
    lab = np.asarray(
        raw3[r3.out_names.index("lab")].addressable_shards[0].data)
    out = np.empty((65536,), np.int32)
    for k in range(NCORES):
        out[8192 * k:8192 * (k + 1)] = lab[k].T.reshape(-1).astype(np.int32)
    return out.reshape(256, 256)


def kernel(**inputs):
    w64 = np.asarray(inputs["weight_64"], np.float32)
    w32 = np.asarray(inputs["weight_32"], np.float32)
    w16 = np.asarray(inputs["weight_16"], np.float32)
    w8 = np.asarray(inputs["weight_8"], np.float32)
    B = w64.shape[0]
    outs = [_segment_one_fused(w64[b], w32[b], w16[b], w8[b])
            for b in range(B)]
    return np.stack(outs).astype(np.int32)

